# revision 1
# baseline (speedup 1.0000x reference)
"""Trainium2 Bass kernel for a DeepseekV2 decoder-layer attention block
(MLA prefill, fp32 reference) distributed across 8 NeuronCores.

Strategy (single NEFF, SPMD on 8 cores):
  - Sequence-shard the shared projections: each core computes q_lora / ckv /
    k_pe (RMS-normed / roped) for its 256 rows of the sequence, in transposed
    layout, then two on-device AllGathers replicate them.
  - Head-shard the rest (4 heads per core): q_b projection + RoPE, kc/vc
    expansion, causal attention (scores computed transposed so the attn@v
    matmul needs no transposes), and a row-shard of w_o.
  - Each core emits a partial [S, HID] output; the host sums the 8 partials
    (the output all-reduce) to produce the full result.

All heavy matmuls run in bf16 with fp32 PSUM accumulation; softmax runs in
fp32 on the scalar engine (no max-subtraction needed: scores are O(3) by
construction and masked lanes underflow exp to exactly 0).

RMS-norm weights, the 1/sqrt(DQ) score scale, and the interleaved->half RoPE
permutation are folded into weights on the host. RoPE rotation is computed
via extra matmuls with sign-flipped, permuted weight columns so every
elementwise op stays partition-aligned.
"""

import numpy as np

S, HID, H = 2048, 5120, 32
QLR, KVLR = 1536, 512
DN, DR, DV = 128, 64, 128
DQ = DN + DR
NC_N = 8
HPC = H // NC_N          # heads per core
SL = S // NC_N           # sequence rows per core (front end)
ROPE_BASE, EPS = 10000.0, 1e-6

_CACHE = {}


def _bf16():
    import ml_dtypes
    return np.dtype(ml_dtypes.bfloat16)


def _build_program():
    import concourse.bass as bass
    import concourse.tile as tile
    from concourse import bacc, mybir
    from contextlib import ExitStack

    f32 = mybir.dt.float32
    bf = mybir.dt.bfloat16
    AF = mybir.ActivationFunctionType

    nc = bacc.Bacc("TRN2", target_bir_lowering=False, debug=False,
                   num_devices=NC_N)

    def din(name, shape, dt=bf):
        return nc.dram_tensor(name, list(shape), dt, kind="ExternalInput").ap()

    hsT_d = din("hsT", (HID, SL))
    wqa_d = din("wqa", (HID, QLR))
    wkvc_d = din("wkvc", (HID, KVLR))
    wkperot_d = din("wkperot", (HID, 256))
    cosl_d = din("cosl", (128, SL))
    sinl_d = din("sinl", (128, SL))
    cosf_d = din("cosf", (128, S))
    sinf_d = din("sinf", (128, S))
    wqb_d = din("wqb", (QLR, 1024))          # nope(4x128) | pe(2x128) | rot(2x128)
    kct_d = din("kct", (HPC * KVLR, DN))     # per head: kc'^T [KVLR, DN]
    vcp_d = din("vcp", (KVLR, HPC * DV))
    wo_d = din("wo", (HPC * DV, HID))
    masks_d = din("masks", (4 * 128, 512))   # additive causal masks (scores^T)
    out_d = nc.dram_tensor("out_partial", [S, HID], f32,
                           kind="ExternalOutput").ap()

    cc1_in = nc.dram_tensor("cc1_in", [KVLR + 128, SL], bf).ap()
    cc1_out = nc.dram_tensor("cc1_out", [NC_N * (KVLR + 128), SL], bf,
                             addr_space="Shared").ap()
    cc2_in = nc.dram_tensor("cc2_in", [QLR, SL], bf).ap()
    cc2a_out = nc.dram_tensor("cc2a_out", [NC_N * (QLR // 2), SL], bf,
                              addr_space="Shared").ap()
    cc2b_out = nc.dram_tensor("cc2b_out", [NC_N * (QLR // 2), SL], bf,
                              addr_space="Shared").ap()

    KH = HID // 128       # 40 k-chunks of the model dim
    KQ = QLR // 128       # 12 chunks of the q-lora dim
    KC = KVLR // 128      # 4 chunks of the kv-lora dim
    SC = S // 512         # 4 sequence chunks of 512
    SB = S // 128         # 16 sequence blocks of 128

    with tile.TileContext(nc) as tc, ExitStack() as ctx:
        def pool(name, bufs):
            return ctx.enter_context(tc.tile_pool(name=name, bufs=bufs))

        p_hs = pool("hs", 10)
        p_w = pool("wstr", 3)
        p_raw = pool("raw", 16)
        p_sq = pool("sqt", 2)
        p_scn = pool("scn", 3)
        p_f32 = pool("fr32", 2)
        p_sml = pool("sml", 8)
        p_one = pool("ones", 2)
        p_cs = pool("cs", 4)
        p_csl = pool("csl", 2)
        p_ckvg = pool("ckvg", 16)
        p_kpeg = pool("kpeg", 4)
        p_qlg = pool("qlg", 12)
        p_K = pool("Kt", 4)
        p_V = pool("Vt", 16)
        p_wqb = pool("wqb", 12)
        p_kc = pool("kc", 16)
        p_vc = pool("vc", 4)
        p_Qn = pool("Qn", 6)
        p_rope = pool("rope", 2)
        p_P = pool("Pt", 3)
        p_oT = pool("oT", 6)
        p_msk = pool("msk", 4)
        p_bc = pool("bc", 2)
        p_wo = pool("wo", 8)
        p_out = pool("outst", 2)

        pp_mm = ctx.enter_context(
            tc.tile_pool(name="pmm", bufs=7, space="PSUM"))
        pp_sm = ctx.enter_context(
            tc.tile_pool(name="psm", bufs=1, space="PSUM"))

        ones_col = p_one.tile([128, 1], bf)       # lhsT for column sums
        nc.vector.memset(ones_col[:], 1.0)
        eps_t = p_one.tile([1, 1], f32, tag="eps", name="eps")
        nc.vector.memset(eps_t[:], EPS)

        # ---------------- FRONT: sequence-sharded projections -------------
        # q_lora pass first (its AllGather is the big one and overlaps the
        # ckv pass + AG1 that follow). 6 accumulation chains per pass-group,
        # one PSUM bank each. hs/wqa stream on the sync HWDGE ring, wkv/wpr
        # on the scalar ring, cc writes + broadcasts + collectives on gpsimd.
        ssq_q = pp_sm.tile([1, SL], f32, tag="sm", name="sm")
        raw_q = []
        for g in range(2):
            ql_ps = [pp_mm.tile([128, SL], f32, tag="mm", name="mm")
                     for _ in range(KQ // 2)]
            for k in range(KH):
                hst = p_hs.tile([128, SL], bf, tag="hs", name="hs")
                nc.sync.dma_start(hst[:], hsT_d[k * 128:(k + 1) * 128, :])
                w = p_w.tile([128, QLR // 2], bf, tag="wqa", name="wqa", bufs=3)
                nc.sync.dma_start(
                    w[:], wqa_d[k * 128:(k + 1) * 128,
                                g * (QLR // 2):(g + 1) * (QLR // 2)])
                for mi in range(KQ // 2):
                    nc.tensor.matmul(ql_ps[mi][:],
                                     w[:, mi * 128:(mi + 1) * 128], hst[:],
                                     start=(k == 0), stop=(k == KH - 1))
            for mi in range(KQ // 2):
                m = g * (KQ // 2) + mi
                r = p_raw.tile([128, SL], bf, tag="raw", name="raw")
                nc.scalar.activation(r[:], ql_ps[mi][:], AF.Copy)
                raw_q.append(r)
                sq = p_sq.tile([128, SL], bf, tag="sq", name="sq")
                nc.scalar.activation(sq[:], ql_ps[mi][:], AF.Square)
                nc.tensor.matmul(ssq_q[:], ones_col[:], sq[:],
                                 start=(m == 0), stop=(m == KQ - 1))
        t_q = p_sml.tile([1, SL], f32, tag="sml", name="sml")
        nc.scalar.activation(t_q[:], ssq_q[:], AF.Sqrt,
                             bias=eps_t[:], scale=1.0 / QLR)
        s_q = p_sml.tile([1, SL], f32, tag="sml", name="sml")
        nc.vector.reciprocal(s_q[:], t_q[:])
        bq_sb = p_bc.tile([128, 512], f32, tag="bc", name="bc")
        nc.gpsimd.partition_broadcast(bq_sb[:, :SL], s_q[:])
        for m in range(KQ):
            qn = p_scn.tile([128, SL], bf, tag="scn", name="scn")
            nc.vector.tensor_mul(qn[:], raw_q[m][:], bq_sb[:, :SL])
            nc.gpsimd.dma_start(cc2_in[m * 128:(m + 1) * 128, :], qn[:])

        nc.gpsimd.collective_compute(
            "AllGather", mybir.AluOpType.bypass,
            ins=[cc2_in[0:QLR // 2, :]], outs=[cc2a_out[:]],
            replica_groups=[list(range(NC_N))],
        )
        nc.gpsimd.collective_compute(
            "AllGather", mybir.AluOpType.bypass,
            ins=[cc2_in[QLR // 2:QLR, :]], outs=[cc2b_out[:]],
            replica_groups=[list(range(NC_N))],
        )

        # --- ckv + k_pe pass ---
        ckv_ps = [pp_mm.tile([128, SL], f32, tag="mm", name="mm")
                  for _ in range(KC)]
        pe_ps = pp_mm.tile([128, SL], f32, tag="mm", name="mm")
        rot_ps = pp_mm.tile([128, SL], f32, tag="mm", name="mm")
        for k in range(KH):
            hst = p_hs.tile([128, SL], bf, tag="hs", name="hs")
            nc.sync.dma_start(hst[:], hsT_d[k * 128:(k + 1) * 128, :])
            wkv = p_w.tile([128, KVLR], bf, tag="wkv", name="wkv", bufs=3)
            nc.scalar.dma_start(wkv[:], wkvc_d[k * 128:(k + 1) * 128, :])
            wpr = p_w.tile([128, 256], bf, tag="wpr", name="wpr", bufs=3)
            nc.scalar.dma_start(wpr[:], wkperot_d[k * 128:(k + 1) * 128, :])
            for c in range(KC):
                nc.tensor.matmul(ckv_ps[c][:], wkv[:, c * 128:(c + 1) * 128],
                                 hst[:], start=(k == 0), stop=(k == KH - 1))
            nc.tensor.matmul(pe_ps[:], wpr[:, 0:128], hst[:],
                             start=(k == 0), stop=(k == KH - 1))
            nc.tensor.matmul(rot_ps[:], wpr[:, 128:256], hst[:],
                             start=(k == 0), stop=(k == KH - 1))
        ssq_kv = pp_sm.tile([1, SL], f32, tag="sm", name="sm")
        raw_kv = []
        for c in range(KC):
            r = p_raw.tile([128, SL], bf, tag="raw", name="raw")
            nc.scalar.activation(r[:], ckv_ps[c][:], AF.Copy)
            raw_kv.append(r)
            sq = p_sq.tile([128, SL], bf, tag="sq", name="sq")
            nc.scalar.activation(sq[:], ckv_ps[c][:], AF.Square)
            nc.tensor.matmul(ssq_kv[:], ones_col[:], sq[:],
                             start=(c == 0), stop=(c == KC - 1))
        t_kv = p_sml.tile([1, SL], f32, tag="sml", name="sml")
        nc.scalar.activation(t_kv[:], ssq_kv[:], AF.Sqrt,
                             bias=eps_t[:], scale=1.0 / KVLR)
        s_kv = p_sml.tile([1, SL], f32, tag="sml", name="sml")
        nc.vector.reciprocal(s_kv[:], t_kv[:])
        bkv_sb = p_bc.tile([128, 512], f32, tag="bc", name="bc")
        nc.gpsimd.partition_broadcast(bkv_sb[:, :SL], s_kv[:])
        for c in range(KC):
            cn = p_scn.tile([128, SL], bf, tag="scn", name="scn")
            nc.vector.tensor_mul(cn[:], raw_kv[c][:], bkv_sb[:, :SL])
            nc.gpsimd.dma_start(cc1_in[c * 128:(c + 1) * 128, :], cn[:])
        # rope k_pe
        cosl_t = p_csl.tile([128, SL], bf, tag="csl", name="csl")
        sinl_t = p_csl.tile([128, SL], bf, tag="csl", name="csl")
        nc.sync.dma_start(cosl_t[:], cosl_d[:, :])
        nc.sync.dma_start(sinl_t[:], sinl_d[:, :])
        t1 = p_f32.tile([128, SL], f32, tag="f32", name="f32")
        t2 = p_f32.tile([128, SL], f32, tag="f32", name="f32")
        nc.vector.tensor_mul(t1[:], pe_ps[:], cosl_t[:])
        nc.vector.tensor_mul(t2[:], rot_ps[:], sinl_t[:])
        kpe_n = p_scn.tile([128, SL], bf, tag="scn", name="scn")
        nc.vector.tensor_add(kpe_n[:], t1[:], t2[:])
        nc.gpsimd.dma_start(cc1_in[KVLR:KVLR + 128, :], kpe_n[:])

        nc.gpsimd.collective_compute(
            "AllGather", mybir.AluOpType.bypass,
            ins=[cc1_in[:]], outs=[cc1_out[:]],
            replica_groups=[list(range(NC_N))],
        )

        # resident back-end weights: prefetch on sync while AGs fly
        kc_t = {}
        for i in range(HPC):
            for c in range(KC):
                t = p_kc.tile([128, DN], bf, tag="kc", name="kc")
                nc.sync.dma_start(
                    t[:], kct_d[i * KVLR + c * 128:i * KVLR + (c + 1) * 128, :])
                kc_t[(i, c)] = t
        vc_t = {}
        for c in range(KC):
            t = p_vc.tile([128, HPC * DV], bf, tag="vc", name="vc")
            nc.sync.dma_start(t[:], vcp_d[c * 128:(c + 1) * 128, :])
            vc_t[c] = t
        wqb_t = []
        for k in range(KQ):
            t = p_wqb.tile([128, 1024], bf, tag="wqb", name="wqb")
            nc.sync.dma_start(t[:], wqb_d[k * 128:(k + 1) * 128, :])
            wqb_t.append(t)
        cosf_t = p_cs.tile([128, S], bf, tag="cs", name="cs")
        sinf_t = p_cs.tile([128, S], bf, tag="cs", name="cs")
        nc.sync.dma_start(cosf_t[:], cosf_d[:, :])
        nc.sync.dma_start(sinf_t[:], sinf_d[:, :])
        mask_t = []
        for m in range(4):
            t = p_msk.tile([128, 512], bf, tag="msk", name="msk")
            nc.sync.dma_start(t[:], masks_d[m * 128:(m + 1) * 128, :])
            mask_t.append(t)

        # ---------------- BACK: head-sharded attention ---------------------
        RPC = 512 // SL     # AG rank-blocks per 512-wide seq chunk
        ckvg = {}
        for c in range(KC):
            for sc in range(SC):
                t = p_ckvg.tile([128, 512], bf, tag="ckvg", name="ckvg")
                for half in range(RPC):
                    r = RPC * sc + half
                    nc.scalar.dma_start(
                        t[:, half * SL:(half + 1) * SL],
                        cc1_out[r * (KVLR + 128) + c * 128:
                                r * (KVLR + 128) + (c + 1) * 128, :])
                ckvg[(c, sc)] = t
        kpeg = {}
        for sc in range(SC):
            t = p_kpeg.tile([128, 512], bf, tag="kpeg", name="kpeg")
            for half in range(RPC):
                r = RPC * sc + half
                nc.scalar.dma_start(
                    t[:, half * SL:(half + 1) * SL],
                    cc1_out[r * (KVLR + 128) + KVLR:
                            r * (KVLR + 128) + KVLR + 128, :])
            kpeg[sc] = t

        # K^T per head: [DN, S]
        K_t = []
        for i in range(HPC):
            kt = p_K.tile([128, S], bf, tag="K", name="K")
            K_t.append(kt)
            for sc in range(SC):
                ps = pp_mm.tile([128, 512], f32, tag="mm", name="mm")
                for c in range(KC):
                    nc.tensor.matmul(ps[:], kc_t[(i, c)][:], ckvg[(c, sc)][:],
                                     start=(c == 0), stop=(c == KC - 1))
                nc.scalar.activation(kt[:, sc * 512:(sc + 1) * 512], ps[:],
                                     AF.Copy)

        # V natural: per seq-block [128, 4*DV]
        V_t = []
        for sb in range(SB):
            ps = pp_mm.tile([128, 512], f32, tag="mm", name="mm")
            for c in range(KC):
                nc.tensor.matmul(
                    ps[:],
                    ckvg[(c, sb // 4)][:, (sb % 4) * 128:(sb % 4 + 1) * 128],
                    vc_t[c][:], start=(c == 0), stop=(c == KC - 1))
            vt = p_V.tile([128, HPC * DV], bf, tag="V", name="V")
            nc.scalar.activation(vt[:], ps[:], AF.Copy)
            V_t.append(vt)

        oT = {}   # (head, sc) -> [DV, 512] bf16 (normalized o transposed)
        for sc in range(SC):
            # gathered q_lora^T tiles for this seq chunk
            qlg = []
            for k in range(KQ):
                t = p_qlg.tile([128, 512], bf, tag="qlg", name="qlg")
                buf = cc2a_out if k < KQ // 2 else cc2b_out
                kk = k if k < KQ // 2 else k - KQ // 2
                for half in range(RPC):
                    r = RPC * sc + half
                    nc.sync.dma_start(
                        t[:, half * SL:(half + 1) * SL],
                        buf[r * (QLR // 2) + kk * 128:
                            r * (QLR // 2) + (kk + 1) * 128, :])
                qlg.append(t)
            # Q^T nope per head (transient)
            qn_t = []
            for i in range(HPC):
                ps = pp_mm.tile([128, 512], f32, tag="mm", name="mm")
                for k in range(KQ):
                    nc.tensor.matmul(ps[:], wqb_t[k][:, i * 128:(i + 1) * 128],
                                     qlg[k][:], start=(k == 0),
                                     stop=(k == KQ - 1))
                qt = p_Qn.tile([128, 512], bf, tag="Qn", name="Qn")
                nc.scalar.activation(qt[:], ps[:], AF.Copy)
                qn_t.append(qt)
            # Q^T pe packs + rope
            roped = []
            for pkt in range(2):
                ps_pe = pp_mm.tile([128, 512], f32, tag="mm", name="mm")
                ps_ro = pp_mm.tile([128, 512], f32, tag="mm", name="mm")
                for k in range(KQ):
                    nc.tensor.matmul(
                        ps_pe[:], wqb_t[k][:, 512 + pkt * 128:512 + (pkt + 1) * 128],
                        qlg[k][:], start=(k == 0), stop=(k == KQ - 1))
                    nc.tensor.matmul(
                        ps_ro[:], wqb_t[k][:, 768 + pkt * 128:768 + (pkt + 1) * 128],
                        qlg[k][:], start=(k == 0), stop=(k == KQ - 1))
                u1 = p_f32.tile([128, 512], f32, tag="rope32", name="rope32")
                u2 = p_f32.tile([128, 512], f32, tag="rope32", name="rope32")
                nc.vector.tensor_mul(u1[:], ps_pe[:],
                                     cosf_t[:, sc * 512:(sc + 1) * 512])
                nc.vector.tensor_mul(u2[:], ps_ro[:],
                                     sinf_t[:, sc * 512:(sc + 1) * 512])
                rp = p_rope.tile([128, 512], bf, tag="rope", name="rope")
                nc.vector.tensor_add(rp[:], u1[:], u2[:])
                roped.append(rp)

            # attention for each head on this seq chunk
            for i in range(HPC):
                pkt, hp = i // 2, i % 2
                o_ps = pp_mm.tile([128, 512], f32, tag="mm", name="mm")
                d_ps = pp_sm.tile([1, 512], f32, tag="sm", name="sm")
                nj = 4 * sc + 4
                for j in range(nj):
                    s_ps = pp_mm.tile([128, 512], f32, tag="mm", name="mm")
                    nc.tensor.matmul(s_ps[:],
                                     K_t[i][:, j * 128:(j + 1) * 128],
                                     qn_t[i][:], start=True, stop=False)
                    nc.tensor.matmul(
                        s_ps[:],
                        kpeg[j // 4][hp * 64:(hp + 1) * 64,
                                     (j % 4) * 128:(j % 4 + 1) * 128],
                        roped[pkt][hp * 64:(hp + 1) * 64, :],
                        start=False, stop=True)
                    pt = p_P.tile([128, 512], bf, tag="P", name="P")
                    if j >= 4 * sc:
                        pr = p_P.tile([128, 512], bf, tag="Praw", name="Praw", bufs=2)
                        nc.scalar.activation(pr[:], s_ps[:], AF.Exp)
                        nc.vector.tensor_mul(pt[:], pr[:],
                                             mask_t[j - 4 * sc][:])
                    else:
                        nc.scalar.activation(pt[:], s_ps[:], AF.Exp)
                    nc.tensor.matmul(d_ps[:], ones_col[:], pt[:],
                                     start=(j == 0), stop=(j == nj - 1))
                    nc.tensor.matmul(o_ps[:],
                                     V_t[j][:, i * DV:(i + 1) * DV], pt[:],
                                     start=(j == 0), stop=(j == nj - 1))
                dinv = p_sml.tile([1, 512], f32, tag="sml", name="sml")
                nc.vector.reciprocal(dinv[:], d_ps[:])
                bc_sb = p_bc.tile([128, 512], f32, tag="bc", name="bc")
                nc.gpsimd.partition_broadcast(bc_sb[:], dinv[:])
                ot = p_oT.tile([128, 512], bf, tag="oT", name="oT")
                nc.vector.tensor_mul(ot[:], o_ps[:], bc_sb[:])
                oT[(i, sc)] = ot

            # w_o partial for this seq chunk (overlaps later chunks' attention)
            for n in range(HID // 512):
                wo_t = []
                for i in range(HPC):
                    t = p_wo.tile([128, 512], bf, tag="wo", name="wo")
                    nc.sync.dma_start(
                        t[:], wo_d[i * DV:(i + 1) * DV, n * 512:(n + 1) * 512])
                    wo_t.append(t)
                for sbl in range(4):
                    sb = sc * 4 + sbl
                    ps = pp_mm.tile([128, 512], f32, tag="mm", name="mm")
                    for i in range(HPC):
                        nc.tensor.matmul(
                            ps[:], oT[(i, sc)][:, sbl * 128:(sbl + 1) * 128],
                            wo_t[i][:], start=(i == 0), stop=(i == HPC - 1))
                    ot2 = p_out.tile([128, 512], f32, tag="outst", name="outst")
                    nc.vector.tensor_copy(ot2[:], ps[:])
                    nc.scalar.dma_start(
                        out_d[sb * 128:(sb + 1) * 128, n * 512:(n + 1) * 512],
                        ot2[:])

    nc.compile()
    return nc


def _prep_inputs(inputs):
    """Host-side sharding + weight folding. Returns in_maps (list of 8 dicts)."""
    BF = _bf16()

    hs = np.asarray(inputs['hidden_states'], np.float32)
    pos = np.asarray(inputs['positions'])
    w_qa = np.asarray(inputs['w_qa'], np.float32)
    q_a_ln_w = np.asarray(inputs['q_a_ln_w'], np.float32)
    w_qb = np.asarray(inputs['w_qb'], np.float32)
    w_kva = np.asarray(inputs['w_kva'], np.float32)
    kv_a_ln_w = np.asarray(inputs['kv_a_ln_w'], np.float32)
    kc = np.asarray(inputs['kc'], np.float32)
    vc = np.asarray(inputs['vc'], np.float32)
    w_o = np.asarray(inputs['w_o'], np.float32)

    perm = np.concatenate([np.arange(0, DR, 2), np.arange(1, DR, 2)])
    inv_freq = 1.0 / (ROPE_BASE ** (np.arange(0, DR, 2, dtype=np.float64) / DR))
    freqs = pos.astype(np.float64)[None, :] * inv_freq[:, None]     # [32, S]
    cosT = np.cos(freqs).astype(np.float32)
    sinT = np.sin(freqs).astype(np.float32)
    cos128 = np.tile(cosT, (4, 1)).astype(BF)                        # [128, S]
    sin128 = np.tile(sinT, (4, 1)).astype(BF)

    scale = DQ ** -0.5
    w_qb_eff = ((w_qb * q_a_ln_w[:, None]) * scale).reshape(QLR, H, DQ)

    w_pe = w_kva[:, KVLR:][:, perm]
    w_pe_rot = np.concatenate([-w_pe[:, 32:], w_pe[:, :32]], 1)
    wkperot = np.concatenate([w_pe, w_pe, w_pe_rot, w_pe_rot],
                             1).astype(BF)                        # [HID, 256]

    kc_f = kc * kv_a_ln_w[None, None, :]
    vc_f = vc * kv_a_ln_w[None, :, None]

    masks = np.ones((4, 128, 512), np.float32)
    for p in range(4):
        for b in range(4):
            blk = masks[p][:, b * 128:(b + 1) * 128]
            if b < p:
                blk[:] = 0.0
            elif b == p:
                kr = np.arange(128)[:, None]
                qc = np.arange(128)[None, :]
                blk[kr > qc] = 0.0
    masks_b = masks.reshape(4 * 128, 512).astype(BF)

    wqa_b = w_qa.astype(BF)
    wkvc_b = w_kva[:, :KVLR].astype(BF)

    in_maps = []
    for core in range(NC_N):
        rows = slice(core * SL, (core + 1) * SL)
        h0 = core * HPC

        wqb_all = np.empty((QLR, 1024), np.float32)
        for i in range(HPC):
            wqb_all[:, i * 128:(i + 1) * 128] = w_qb_eff[:, h0 + i, :DN]
        for pkt in range(2):
            a, b = h0 + 2 * pkt, h0 + 2 * pkt + 1
            pe_a = w_qb_eff[:, a, DN:][:, perm]
            pe_b = w_qb_eff[:, b, DN:][:, perm]
            wqb_all[:, 512 + pkt * 128:512 + pkt * 128 + 64] = pe_a
            wqb_all[:, 512 + pkt * 128 + 64:512 + (pkt + 1) * 128] = pe_b
            rot_a = np.concatenate([-pe_a[:, 32:], pe_a[:, :32]], 1)
            rot_b = np.concatenate([-pe_b[:, 32:], pe_b[:, :32]], 1)
            wqb_all[:, 768 + pkt * 128:768 + pkt * 128 + 64] = rot_a
            wqb_all[:, 768 + pkt * 128 + 64:768 + (pkt + 1) * 128] = rot_b

        kct = np.concatenate([kc_f[h0 + i].T for i in range(HPC)], 0)
        vcp = np.concatenate([vc_f[h0 + i] for i in range(HPC)], 1)
        wo_sh = w_o[h0 * DV:(h0 + HPC) * DV, :]

        in_maps.append({
            "hsT": np.ascontiguousarray(hs[rows].T).astype(BF),
            "wqa": wqa_b,
            "wkvc": wkvc_b,
            "wkperot": wkperot,
            "cosl": np.ascontiguousarray(cos128[:, rows]),
            "sinl": np.ascontiguousarray(sin128[:, rows]),
            "cosf": cos128,
            "sinf": sin128,
            "wqb": wqb_all.astype(BF),
            "kct": kct.astype(BF),
            "vcp": vcp.astype(BF),
            "wo": wo_sh.astype(BF),
            "masks": masks_b,
        })
    return in_maps


def _get_program():
    if "nc" not in _CACHE:
        _CACHE["nc"] = _build_program()
    return _CACHE["nc"]


def run(inputs, trace=False, trace_kwargs=None):
    """Build (cached), run on 8 cores, return (output, BassKernelResults)."""
    from concourse.bass_utils import run_bass_kernel_spmd

    nc = _get_program()
    in_maps = _prep_inputs(inputs)
    res = run_bass_kernel_spmd(nc, in_maps, list(range(NC_N)),
                               trace=trace, **(trace_kwargs or {}))
    out = np.zeros((S, HID), np.float32)
    for r in res.results:
        out += r["out_partial"]
    return out, res


def kernel(**inputs) -> np.ndarray:
    out, _ = run(inputs, trace=False)
    return out



# revision 10
# speedup vs baseline: 1.0844x; 1.0844x over previous
"""Trainium2 Bass kernel for a DeepseekV2 decoder-layer attention block
(MLA prefill, fp32 reference) distributed across 8 NeuronCores.

Strategy (single NEFF, SPMD on 8 cores):
  - Sequence-shard the shared projections: each core computes ckv / k_pe
    (RMS-normed / roped) then q_lora for its 256 rows of the sequence, in
    transposed layout; on-device AllGathers replicate them (ckv AG first so
    the K/V expansion overlaps the q AllGathers).
  - Head-shard the rest (4 heads per core): q_b projection + RoPE, kc/vc
    expansion, causal attention (scores computed transposed so the attn@v
    matmul needs no transposes), and a row-shard of w_o.
  - Each core emits a partial [S, HID] bf16 output; the host sums the 8
    partials (the output all-reduce) in fp32.

Tensor-engine economies vs the naive formulation:
  - RoPE "rotate-half" is a partition permutation: done with 4 small
    SBUF->SBUF DMA row swaps + sign-folded sin tiles instead of duplicate
    sign-flipped weight-matmul chains.
  - softmax denominators accumulate on the vector engine (exp tiles summed
    across key blocks); one [1,512] ones-matmul per (head, chunk) finishes
    the partition reduction.
  - causal diagonal blocks compute only the unmasked column range; the
    triangle mask is applied additively (-1e30) on PSUM before exp.
  - reciprocals run after a partition-broadcast so all 128 vector lanes
    work; w_o stays SBUF-resident; outputs staged/written as bf16.
"""

import numpy as np

S, HID, H = 2048, 5120, 32
QLR, KVLR = 1536, 512
DN, DR, DV = 128, 64, 128
DQ = DN + DR
NC_N = 8
HPC = H // NC_N          # heads per core
SL = S // NC_N           # sequence rows per core (front end)
ROPE_BASE, EPS = 10000.0, 1e-6

_CACHE = {}


def _bf16():
    import ml_dtypes
    return np.dtype(ml_dtypes.bfloat16)


def _build_program():
    import concourse.bass as bass
    import concourse.tile as tile
    from concourse import bacc, mybir
    from contextlib import ExitStack

    f32 = mybir.dt.float32
    bf = mybir.dt.bfloat16
    AF = mybir.ActivationFunctionType

    nc = bacc.Bacc("TRN2", target_bir_lowering=False, debug=False,
                   num_devices=NC_N)

    def din(name, shape, dt=bf):
        return nc.dram_tensor(name, list(shape), dt, kind="ExternalInput").ap()

    hsT_d = din("hsT", (HID, SL))
    wqa_d = din("wqa", (HID, QLR))
    wkvk_d = din("wkvk", (HID, KVLR + 128))   # ckv cols | pe-dup cols
    cosl_d = din("cosl", (128, SL))
    sinl_d = din("sinl", (128, SL))           # sign-folded
    cosf_d = din("cosf", (128, S))
    sinf_d = din("sinf", (128, S))            # sign-folded
    wqb_d = din("wqb", (QLR, 768))            # nope(4x128) | pe(2x128)
    kct_d = din("kct", (HPC * KVLR, DN))      # per head: kc'^T [KVLR, DN]
    vcp_d = din("vcp", (KVLR, HPC * DV))
    wo_d = din("wo", (HPC * DV, HID))
    tri_d = din("tri", (128, 128))            # additive causal mask (scores^T)
    out_d = nc.dram_tensor("out_partial", [S, HID], bf,
                           kind="ExternalOutput").ap()

    cc1_in = nc.dram_tensor("cc1_in", [KVLR + 128, SL], bf).ap()
    cc1_out = nc.dram_tensor("cc1_out", [NC_N * (KVLR + 128), SL], bf,
                             addr_space="Shared").ap()
    cc2_in = nc.dram_tensor("cc2_in", [QLR, SL], bf).ap()
    cc2a_out = nc.dram_tensor("cc2a_out", [NC_N * (QLR // 2), SL], bf,
                              addr_space="Shared").ap()
    cc2b_out = nc.dram_tensor("cc2b_out", [NC_N * (QLR // 2), SL], bf,
                              addr_space="Shared").ap()

    KH = HID // 128       # 40 k-chunks of the model dim
    KQ = QLR // 128       # 12 chunks of the q-lora dim
    KC = KVLR // 128      # 4 chunks of the kv-lora dim
    SC = S // 512         # 4 sequence chunks of 512
    SB = S // 128         # 16 sequence blocks of 128
    RPC = 512 // SL       # AG rank-blocks per 512-wide seq chunk

    with tile.TileContext(nc) as tc, ExitStack() as ctx:
        def pool(name, bufs, where=ctx):
            return where.enter_context(tc.tile_pool(name=name, bufs=bufs))

        # ---- persistent pools (live whole program) ----
        p_one = pool("ones", 2)
        p_wqb = pool("wqb", 12)
        p_wo = pool("wo", 4)
        p_cs = pool("cs", 4)
        p_K = pool("Kt", 4)
        p_V = pool("Vt", 16)
        p_kpeg = pool("kpeg", 4)
        p_sml = pool("sml", 3)
        p_qlg = pool("qlg", 12)
        p_Qn = pool("Qn", 5)
        p_pe = pool("pe", 4)
        p_f32 = pool("fr32", 2)
        p_rope = pool("rope", 4)
        p_P = pool("Pt", 3)
        p_acc = pool("acc", 2)
        p_oT = pool("oT", 5)
        p_bc = pool("bc", 3)
        p_msk = pool("msk", 1)
        p_out = pool("outst", 2)

        pp_mm = ctx.enter_context(
            tc.tile_pool(name="pmm", bufs=7, space="PSUM"))
        pp_sm = ctx.enter_context(
            tc.tile_pool(name="psm", bufs=1, space="PSUM"))

        ones_col = p_one.tile([128, 1], bf)       # lhsT for column sums
        nc.vector.memset(ones_col[:], 1.0)
        ones_f32 = p_one.tile([128, 1], f32, tag="onesf", name="onesf")
        nc.vector.memset(ones_f32[:], 1.0)
        eps_t = p_one.tile([1, 1], f32, tag="eps", name="eps")
        nc.vector.memset(eps_t[:], EPS)

        with ExitStack() as fctx:
            # ---- front-phase pools (released before attention) ----
            p_hs = pool("hs", 6, fctx)
            p_w = pool("wstr", 3, fctx)
            p_raw = pool("raw", 6, fctx)
            p_sq = pool("sqt", 2, fctx)
            p_scn = pool("scn", 3, fctx)
            p_csl = pool("csl", 2, fctx)
            p_ckvg = pool("ckvg", 16, fctx)
            p_kc = pool("kc", 16, fctx)
            p_vc = pool("vc", 4, fctx)

            # ------------- FRONT 1: ckv + k_pe pass (AG first) -------------
            ckv_ps = [pp_mm.tile([128, SL], f32, tag="mm", name="mm")
                      for _ in range(KC)]
            pe_ps = pp_mm.tile([128, SL], f32, tag="mm", name="mm")
            for k in range(KH):
                hst = p_hs.tile([128, SL], bf, tag="hs", name="hs")
                nc.sync.dma_start(hst[:], hsT_d[k * 128:(k + 1) * 128, :])
                wkv = p_w.tile([128, KVLR + 128], bf, tag="wkv", name="wkv")
                nc.scalar.dma_start(wkv[:], wkvk_d[k * 128:(k + 1) * 128, :])
                for c in range(KC):
                    nc.tensor.matmul(ckv_ps[c][:],
                                     wkv[:, c * 128:(c + 1) * 128],
                                     hst[:], start=(k == 0), stop=(k == KH - 1))
                nc.tensor.matmul(pe_ps[:], wkv[:, KVLR:KVLR + 128], hst[:],
                                 start=(k == 0), stop=(k == KH - 1))
            ssq_kv = pp_sm.tile([1, SL], f32, tag="sm", name="sm")
            for c in range(KC):
                sq = p_sq.tile([128, SL], bf, tag="sq", name="sq")
                nc.scalar.activation(sq[:], ckv_ps[c][:], AF.Square)
                nc.tensor.matmul(ssq_kv[:], ones_col[:], sq[:],
                                 start=(c == 0), stop=(c == KC - 1))
            t_kv = p_sml.tile([1, SL], f32, tag="sml", name="sml")
            nc.scalar.activation(t_kv[:], ssq_kv[:], AF.Sqrt,
                                 bias=eps_t[:], scale=1.0 / KVLR)
            bkv = p_bc.tile([128, 512], f32, tag="bc", name="bc")
            nc.gpsimd.partition_broadcast(bkv[:, :SL], t_kv[:])
            rkv = p_bc.tile([128, 512], f32, tag="bc", name="bc")
            nc.vector.reciprocal(rkv[:, :SL], bkv[:, :SL])
            for c in range(KC):
                cn = p_scn.tile([128, SL], bf, tag="scn", name="scn")
                nc.vector.tensor_mul(cn[:], ckv_ps[c][:], rkv[:, :SL])
                nc.gpsimd.dma_start(cc1_in[c * 128:(c + 1) * 128, :], cn[:])
            # k_pe rope: rot = partition swap of pe (sign folded into sinl)
            cosl_t = p_csl.tile([128, SL], bf, tag="csl", name="csl")
            sinl_t = p_csl.tile([128, SL], bf, tag="csl", name="csl")
            nc.sync.dma_start(cosl_t[:], cosl_d[:, :])
            nc.sync.dma_start(sinl_t[:], sinl_d[:, :])
            pe_sb = p_scn.tile([128, SL], bf, tag="scn", name="scn")
            nc.scalar.activation(pe_sb[:], pe_ps[:], AF.Copy)
            rot_sb = p_scn.tile([128, SL], bf, tag="scn", name="scn")
            for h in range(4):
                src = (h ^ 1) * 32
                nc.gpsimd.dma_start(rot_sb[h * 32:(h + 1) * 32, :],
                                    pe_sb[src:src + 32, :])
            t1 = p_f32.tile([128, SL], f32, tag="f32", name="f32")
            t2 = p_f32.tile([128, SL], f32, tag="f32", name="f32")
            nc.vector.tensor_mul(t1[:], pe_ps[:], cosl_t[:])
            nc.vector.tensor_mul(t2[:], rot_sb[:], sinl_t[:])
            kpe_n = p_scn.tile([128, SL], bf, tag="scn", name="scn")
            nc.vector.tensor_add(kpe_n[:], t1[:], t2[:])
            nc.gpsimd.dma_start(cc1_in[KVLR:KVLR + 128, :], kpe_n[:])

            nc.gpsimd.collective_compute(
                "AllGather", mybir.AluOpType.bypass,
                ins=[cc1_in[:]], outs=[cc1_out[:]],
                replica_groups=[list(range(NC_N))],
            )

            # ------------- FRONT 2: q_lora pass -------------
            ssq_q = pp_sm.tile([1, SL], f32, tag="sm", name="sm")
            raw_q = []
            g1_ps = None
            for g in range(2):
                ql_ps = [pp_mm.tile([128, SL], f32, tag="mm", name="mm")
                         for _ in range(KQ // 2)]
                for k in range(KH):
                    hst = p_hs.tile([128, SL], bf, tag="hs", name="hs")
                    nc.sync.dma_start(hst[:], hsT_d[k * 128:(k + 1) * 128, :])
                    w = p_w.tile([128, QLR // 2], bf, tag="wqa", name="wqa")
                    nc.sync.dma_start(
                        w[:], wqa_d[k * 128:(k + 1) * 128,
                                    g * (QLR // 2):(g + 1) * (QLR // 2)])
                    for mi in range(KQ // 2):
                        nc.tensor.matmul(ql_ps[mi][:],
                                         w[:, mi * 128:(mi + 1) * 128], hst[:],
                                         start=(k == 0), stop=(k == KH - 1))
                for mi in range(KQ // 2):
                    m = g * (KQ // 2) + mi
                    sq = p_sq.tile([128, SL], bf, tag="sq", name="sq")
                    nc.scalar.activation(sq[:], ql_ps[mi][:], AF.Square)
                    nc.tensor.matmul(ssq_q[:], ones_col[:], sq[:],
                                     start=(m == 0), stop=(m == KQ - 1))
                    if g == 0:
                        r = p_raw.tile([128, SL], bf, tag="raw", name="raw")
                        nc.scalar.activation(r[:], ql_ps[mi][:], AF.Copy)
                        raw_q.append(r)
                if g == 1:
                    g1_ps = ql_ps
            t_q = p_sml.tile([1, SL], f32, tag="sml", name="sml")
            nc.scalar.activation(t_q[:], ssq_q[:], AF.Sqrt,
                                 bias=eps_t[:], scale=1.0 / QLR)
            bq = p_bc.tile([128, 512], f32, tag="bc", name="bc")
            nc.gpsimd.partition_broadcast(bq[:, :SL], t_q[:])
            rq = p_bc.tile([128, 512], f32, tag="bc", name="bc")
            nc.vector.reciprocal(rq[:, :SL], bq[:, :SL])
            for m in range(KQ):
                qn = p_scn.tile([128, SL], bf, tag="scn", name="scn")
                src = raw_q[m][:] if m < KQ // 2 else g1_ps[m - KQ // 2][:]
                nc.vector.tensor_mul(qn[:], src, rq[:, :SL])
                nc.gpsimd.dma_start(cc2_in[m * 128:(m + 1) * 128, :], qn[:])

            nc.gpsimd.collective_compute(
                "AllGather", mybir.AluOpType.bypass,
                ins=[cc2_in[0:QLR // 2, :]], outs=[cc2a_out[:]],
                replica_groups=[list(range(NC_N))],
            )
            nc.gpsimd.collective_compute(
                "AllGather", mybir.AluOpType.bypass,
                ins=[cc2_in[QLR // 2:QLR, :]], outs=[cc2b_out[:]],
                replica_groups=[list(range(NC_N))],
            )

            # resident back-end weights: prefetch on sync while AGs fly
            wqb_t = []
            for k in range(KQ):
                t = p_wqb.tile([128, 768], bf, tag="wqb", name="wqb")
                nc.sync.dma_start(t[:], wqb_d[k * 128:(k + 1) * 128, :])
                wqb_t.append(t)
            kc_t = {}
            for i in range(HPC):
                for c in range(KC):
                    t = p_kc.tile([128, DN], bf, tag="kc", name="kc")
                    nc.sync.dma_start(
                        t[:],
                        kct_d[i * KVLR + c * 128:i * KVLR + (c + 1) * 128, :])
                    kc_t[(i, c)] = t
            vc_t = {}
            for c in range(KC):
                t = p_vc.tile([128, HPC * DV], bf, tag="vc", name="vc")
                nc.sync.dma_start(t[:], vcp_d[c * 128:(c + 1) * 128, :])
                vc_t[c] = t
            tri_t = p_msk.tile([128, 128], bf, tag="msk", name="msk")
            nc.sync.dma_start(tri_t[:], tri_d[:, :])
            wo_t = []
            for i in range(HPC):
                t = p_wo.tile([128, HID], bf, tag="wo", name="wo")
                nc.sync.dma_start(t[:], wo_d[i * DV:(i + 1) * DV, :])
                wo_t.append(t)

            # gathered ckv / kpe (scalar ring, waits on AG1)
            ckvg = {}
            for c in range(KC):
                for sc in range(SC):
                    t = p_ckvg.tile([128, 512], bf, tag="ckvg", name="ckvg")
                    for half in range(RPC):
                        r = RPC * sc + half
                        nc.scalar.dma_start(
                            t[:, half * SL:(half + 1) * SL],
                            cc1_out[r * (KVLR + 128) + c * 128:
                                    r * (KVLR + 128) + (c + 1) * 128, :])
                    ckvg[(c, sc)] = t
            kpeg = {}
            for sc in range(SC):
                t = p_kpeg.tile([128, 512], bf, tag="kpeg", name="kpeg")
                for half in range(RPC):
                    r = RPC * sc + half
                    nc.scalar.dma_start(
                        t[:, half * SL:(half + 1) * SL],
                        cc1_out[r * (KVLR + 128) + KVLR:
                                r * (KVLR + 128) + KVLR + 128, :])
                kpeg[sc] = t

            # K^T per head: [DN, S] — overlaps the q AllGathers
            K_t = []
            for i in range(HPC):
                kt = p_K.tile([128, S], bf, tag="K", name="K")
                K_t.append(kt)
                for sc in range(SC):
                    ps = pp_mm.tile([128, 512], f32, tag="mm", name="mm")
                    for c in range(KC):
                        nc.tensor.matmul(ps[:], kc_t[(i, c)][:],
                                         ckvg[(c, sc)][:],
                                         start=(c == 0), stop=(c == KC - 1))
                    nc.scalar.activation(kt[:, sc * 512:(sc + 1) * 512],
                                         ps[:], AF.Copy)

            # V natural: per seq-block [128, 4*DV]
            V_t = []
            for sb in range(SB):
                ps = pp_mm.tile([128, 512], f32, tag="mm", name="mm")
                for c in range(KC):
                    nc.tensor.matmul(
                        ps[:],
                        ckvg[(c, sb // 4)][:, (sb % 4) * 128:(sb % 4 + 1) * 128],
                        vc_t[c][:], start=(c == 0), stop=(c == KC - 1))
                vt = p_V.tile([128, HPC * DV], bf, tag="V", name="V")
                nc.scalar.activation(vt[:], ps[:], AF.Copy)
                V_t.append(vt)
        # ---- front pools released here ----

        # ---------------- BACK: head-sharded attention ---------------------
        for sc in range(SC):
            # gathered q_lora^T tiles for this seq chunk (a-half first)
            qlg = []
            for k in range(KQ):
                t = p_qlg.tile([128, 512], bf, tag="qlg", name="qlg")
                buf = cc2a_out if k < KQ // 2 else cc2b_out
                kk = k if k < KQ // 2 else k - KQ // 2
                for half in range(RPC):
                    r = RPC * sc + half
                    nc.sync.dma_start(
                        t[:, half * SL:(half + 1) * SL],
                        buf[r * (QLR // 2) + kk * 128:
                            r * (QLR // 2) + (kk + 1) * 128, :])
                qlg.append(t)
            cosf_t = p_cs.tile([128, 512], bf, tag="cs", name="cs")
            sinf_t = p_cs.tile([128, 512], bf, tag="cs", name="cs")
            nc.sync.dma_start(cosf_t[:], cosf_d[:, sc * 512:(sc + 1) * 512])
            nc.sync.dma_start(sinf_t[:], sinf_d[:, sc * 512:(sc + 1) * 512])
            # Q^T nope per head (transient)
            qn_t = []
            for i in range(HPC):
                ps = pp_mm.tile([128, 512], f32, tag="mm", name="mm")
                for k in range(KQ):
                    nc.tensor.matmul(ps[:], wqb_t[k][:, i * 128:(i + 1) * 128],
                                     qlg[k][:], start=(k == 0),
                                     stop=(k == KQ - 1))
                qt = p_Qn.tile([128, 512], bf, tag="Qn", name="Qn")
                nc.scalar.activation(qt[:], ps[:], AF.Copy)
                qn_t.append(qt)
            # Q^T pe packs + rope (rot = partition swap, sign in sinf)
            roped = []
            for pkt in range(2):
                ps_pe = pp_mm.tile([128, 512], f32, tag="mm", name="mm")
                for k in range(KQ):
                    nc.tensor.matmul(
                        ps_pe[:], wqb_t[k][:, 512 + pkt * 128:512 + (pkt + 1) * 128],
                        qlg[k][:], start=(k == 0), stop=(k == KQ - 1))
                qpe_sb = p_pe.tile([128, 512], bf, tag="pe", name="pe")
                nc.scalar.activation(qpe_sb[:], ps_pe[:], AF.Copy)
                qrot_sb = p_pe.tile([128, 512], bf, tag="pe", name="pe")
                for h in range(4):
                    src = (h ^ 1) * 32
                    nc.gpsimd.dma_start(qrot_sb[h * 32:(h + 1) * 32, :],
                                        qpe_sb[src:src + 32, :])
                u1 = p_f32.tile([128, 512], f32, tag="rope32", name="rope32")
                u2 = p_f32.tile([128, 512], f32, tag="rope32", name="rope32")
                nc.vector.tensor_mul(u1[:], ps_pe[:], cosf_t[:])
                nc.vector.tensor_mul(u2[:], qrot_sb[:], sinf_t[:])
                rp = p_rope.tile([128, 512], bf, tag="rope", name="rope")
                nc.vector.tensor_add(rp[:], u1[:], u2[:])
                roped.append(rp)

            # attention for each head on this seq chunk
            oT = {}
            for i in range(HPC):
                pkt, hp = i // 2, i % 2
                o_ps = pp_mm.tile([128, 512], f32, tag="mm", name="mm")
                acc = p_acc.tile([128, 512], f32, tag="acc", name="acc")
                nj = 4 * sc + 4
                for j in range(nj):
                    lo = (j - 4 * sc) * 128 if j >= 4 * sc else 0
                    s_ps = pp_mm.tile([128, 512], f32, tag="mm", name="mm")
                    nc.tensor.matmul(s_ps[:, lo:512],
                                     K_t[i][:, j * 128:(j + 1) * 128],
                                     qn_t[i][:, lo:512],
                                     start=True, stop=False)
                    nc.tensor.matmul(
                        s_ps[:, lo:512],
                        kpeg[j // 4][hp * 64:(hp + 1) * 64,
                                     (j % 4) * 128:(j % 4 + 1) * 128],
                        roped[pkt][hp * 64:(hp + 1) * 64, lo:512],
                        start=False, stop=True)
                    if j >= 4 * sc:
                        nc.vector.tensor_add(s_ps[:, lo:lo + 128],
                                             s_ps[:, lo:lo + 128], tri_t[:])
                    pt = p_P.tile([128, 512], bf, tag="P", name="P")
                    nc.scalar.activation(pt[:, lo:512], s_ps[:, lo:512],
                                         AF.Exp)
                    if j == 0:
                        nc.vector.tensor_copy(acc[:], pt[:])
                    else:
                        nc.vector.tensor_add(acc[:, lo:512], acc[:, lo:512],
                                             pt[:, lo:512])
                    nc.tensor.matmul(o_ps[:, lo:512],
                                     V_t[j][:, i * DV:(i + 1) * DV],
                                     pt[:, lo:512],
                                     start=(j == 0), stop=(j == nj - 1),
                                     skip_group_check=True)
                d_ps = pp_sm.tile([1, 512], f32, tag="sm", name="sm")
                nc.tensor.matmul(d_ps[:], ones_f32[:], acc[:],
                                 start=True, stop=True)
                d_sb = p_sml.tile([1, 512], f32, tag="sml", name="sml")
                nc.scalar.activation(d_sb[:], d_ps[:], AF.Copy)
                dbc = p_bc.tile([128, 512], f32, tag="bc", name="bc")
                nc.gpsimd.partition_broadcast(dbc[:], d_sb[:])
                dinv = p_bc.tile([128, 512], f32, tag="bc", name="bc")
                nc.vector.reciprocal(dinv[:], dbc[:])
                ot = p_oT.tile([128, 512], bf, tag="oT", name="oT")
                nc.vector.tensor_mul(ot[:], o_ps[:], dinv[:])
                oT[i] = ot

            # w_o partial for this seq chunk (overlaps later chunks)
            for sbl in range(4):
                sb = sc * 4 + sbl
                stage = p_out.tile([128, HID // 2], bf, tag="outst",
                                   name="outst")
                for n in range(HID // 512):
                    ps = pp_mm.tile([128, 512], f32, tag="mm", name="mm")
                    for i in range(HPC):
                        nc.tensor.matmul(
                            ps[:], oT[i][:, sbl * 128:(sbl + 1) * 128],
                            wo_t[i][:, n * 512:(n + 1) * 512],
                            start=(i == 0), stop=(i == HPC - 1))
                    half, off = n // 5, (n % 5) * 512
                    nc.vector.tensor_copy(stage[:, off:off + 512], ps[:])
                    if n % 5 == 4:
                        nc.gpsimd.dma_start(
                            out_d[sb * 128:(sb + 1) * 128,
                                  half * 2560:(half + 1) * 2560],
                            stage[:, :])
                        if n == 4:
                            stage = p_out.tile([128, HID // 2], bf,
                                               tag="outst", name="outst")

    nc.compile()
    return nc


def _prep_inputs(inputs):
    """Host-side sharding + weight folding. Returns in_maps (list of 8 dicts)."""
    BF = _bf16()

    hs = np.asarray(inputs['hidden_states'], np.float32)
    pos = np.asarray(inputs['positions'])
    w_qa = np.asarray(inputs['w_qa'], np.float32)
    q_a_ln_w = np.asarray(inputs['q_a_ln_w'], np.float32)
    w_qb = np.asarray(inputs['w_qb'], np.float32)
    w_kva = np.asarray(inputs['w_kva'], np.float32)
    kv_a_ln_w = np.asarray(inputs['kv_a_ln_w'], np.float32)
    kc = np.asarray(inputs['kc'], np.float32)
    vc = np.asarray(inputs['vc'], np.float32)
    w_o = np.asarray(inputs['w_o'], np.float32)

    perm = np.concatenate([np.arange(0, DR, 2), np.arange(1, DR, 2)])
    inv_freq = 1.0 / (ROPE_BASE ** (np.arange(0, DR, 2, dtype=np.float64) / DR))
    freqs = pos.astype(np.float64)[None, :] * inv_freq[:, None]     # [32, S]
    cosT = np.cos(freqs).astype(np.float32)
    sinT = np.sin(freqs).astype(np.float32)
    cos128 = np.tile(cosT, (4, 1)).astype(BF)                        # [128, S]
    # rot rows carry the swapped halves; the rotation sign lives here:
    # rows 0-31 multiply -sin, rows 32-63 multiply +sin (per 64-block).
    sin128s = np.tile(np.concatenate([-sinT, sinT], 0), (2, 1)).astype(BF)

    scale = DQ ** -0.5
    w_qb_eff = ((w_qb * q_a_ln_w[:, None]) * scale).reshape(QLR, H, DQ)

    w_pe = w_kva[:, KVLR:][:, perm]
    wkvk = np.concatenate([w_kva[:, :KVLR], w_pe, w_pe], 1).astype(BF)

    kc_f = kc * kv_a_ln_w[None, None, :]
    vc_f = vc * kv_a_ln_w[None, :, None]

    kr = np.arange(128)[:, None]
    qc = np.arange(128)[None, :]
    tri = np.where(kr > qc, -1e30, 0.0).astype(np.float32).astype(BF)

    wqa_b = w_qa.astype(BF)

    in_maps = []
    for core in range(NC_N):
        rows = slice(core * SL, (core + 1) * SL)
        h0 = core * HPC

        wqb_all = np.empty((QLR, 768), np.float32)
        for i in range(HPC):
            wqb_all[:, i * 128:(i + 1) * 128] = w_qb_eff[:, h0 + i, :DN]
        for pkt in range(2):
            a, b = h0 + 2 * pkt, h0 + 2 * pkt + 1
            pe_a = w_qb_eff[:, a, DN:][:, perm]
            pe_b = w_qb_eff[:, b, DN:][:, perm]
            wqb_all[:, 512 + pkt * 128:512 + pkt * 128 + 64] = pe_a
            wqb_all[:, 512 + pkt * 128 + 64:512 + (pkt + 1) * 128] = pe_b

        kct = np.concatenate([kc_f[h0 + i].T for i in range(HPC)], 0)
        vcp = np.concatenate([vc_f[h0 + i] for i in range(HPC)], 1)
        wo_sh = w_o[h0 * DV:(h0 + HPC) * DV, :]

        in_maps.append({
            "hsT": np.ascontiguousarray(hs[rows].T).astype(BF),
            "wqa": wqa_b,
            "wkvk": wkvk,
            "cosl": np.ascontiguousarray(cos128[:, rows]),
            "sinl": np.ascontiguousarray(sin128s[:, rows]),
            "cosf": cos128,
            "sinf": sin128s,
            "wqb": wqb_all.astype(BF),
            "kct": kct.astype(BF),
            "vcp": vcp.astype(BF),
            "wo": wo_sh.astype(BF),
            "tri": tri,
        })
    return in_maps


def _get_program():
    if "nc" not in _CACHE:
        _CACHE["nc"] = _build_program()
    return _CACHE["nc"]


def run(inputs, trace=False, trace_kwargs=None):
    """Build (cached), run on 8 cores, return (output, BassKernelResults)."""
    from concourse.bass_utils import run_bass_kernel_spmd

    nc = _get_program()
    in_maps = _prep_inputs(inputs)
    res = run_bass_kernel_spmd(nc, in_maps, list(range(NC_N)),
                               trace=trace, **(trace_kwargs or {}))
    out = np.zeros((S, HID), np.float32)
    for r in res.results:
        out += r["out_partial"].astype(np.float32)
    return out, res


def kernel(**inputs) -> np.ndarray:
    out, _ = run(inputs, trace=False)
    return out


# revision 23
# speedup vs baseline: 1.1882x; 1.0958x over previous
"""Trainium2 Bass kernel for a DeepseekV2 decoder-layer attention block
(MLA prefill, fp32 reference) distributed across 8 NeuronCores.

Strategy (single NEFF, SPMD on 8 cores):
  - Sequence-shard the shared projections: each core computes ckv / k_pe
    (RMS-normed / roped) then q_lora for its 256 rows of the sequence, in
    transposed layout; on-device AllGathers replicate them (ckv AG first so
    the K/V expansion overlaps the q AllGathers).
  - Head-shard the rest (4 heads per core): q_b projection + RoPE, kc/vc
    expansion, causal attention (scores computed transposed so the attn@v
    matmul needs no transposes), and a row-shard of w_o.
  - Each core emits a partial [S, HID] bf16 output; the host sums the 8
    partials (the output all-reduce) in fp32.

Tensor-engine economies vs the naive formulation:
  - RoPE "rotate-half" is a partition permutation: done with 4 small
    SBUF->SBUF DMA row swaps + sign-folded sin tiles instead of duplicate
    sign-flipped weight-matmul chains.
  - softmax denominators accumulate on the vector engine (exp tiles summed
    across key blocks); one [1,512] ones-matmul per (head, chunk) finishes
    the partition reduction.
  - causal diagonal blocks compute only the unmasked column range; the
    triangle mask is applied additively (-1e30) on PSUM before exp.
  - reciprocals run after a partition-broadcast so all 128 vector lanes
    work; w_o stays SBUF-resident; outputs staged/written as bf16.
"""

import numpy as np

S, HID, H = 2048, 5120, 32
QLR, KVLR = 1536, 512
DN, DR, DV = 128, 64, 128
DQ = DN + DR
NC_N = 8
HPC = H // NC_N          # heads per core
SL = S // NC_N           # sequence rows per core (front end)
ROPE_BASE, EPS = 10000.0, 1e-6

_CACHE = {}


def _bf16():
    import ml_dtypes
    return np.dtype(ml_dtypes.bfloat16)


def _build_program():
    import concourse.bass as bass
    import concourse.tile as tile
    from concourse import bacc, mybir
    from contextlib import ExitStack

    f32 = mybir.dt.float32
    bf = mybir.dt.bfloat16
    AF = mybir.ActivationFunctionType

    nc = bacc.Bacc("TRN2", target_bir_lowering=False, debug=False,
                   num_devices=NC_N)

    def din(name, shape, dt=bf):
        return nc.dram_tensor(name, list(shape), dt, kind="ExternalInput").ap()

    hsT_d = din("hsT", (HID, SL))
    wqa_d = din("wqa", (HID, QLR))
    wkvk_d = din("wkvk", (HID, KVLR + 128))   # ckv cols | pe-dup cols
    cosl_d = din("cosl", (128, SL))
    sinl_d = din("sinl", (128, SL))           # sign-folded
    cosf_d = din("cosf", (128, S))
    sinf_d = din("sinf", (128, S))            # sign-folded
    wqb_d = din("wqb", (QLR, 768))            # nope(4x128) | pe(2x128)
    kct_d = din("kct", (HPC * KVLR, DN))      # per head: kc'^T [KVLR, DN]
    vcp_d = din("vcp", (KVLR, HPC * DV))
    wo_d = din("wo", (HPC * DV, HID))
    tri_d = din("tri", (128, 128))            # additive causal mask (scores^T)
    out_d = nc.dram_tensor("out_partial", [S, HID], bf,
                           kind="ExternalOutput").ap()

    cc1_in = nc.dram_tensor("cc1_in", [KVLR + 128, SL], bf).ap()
    cc1_out = nc.dram_tensor("cc1_out", [NC_N * (KVLR + 128), SL], bf,
                             addr_space="Shared").ap()
    cc2_in = nc.dram_tensor("cc2_in", [QLR, SL], bf).ap()
    cc2a_out = nc.dram_tensor("cc2a_out", [NC_N * (QLR // 2), SL], bf,
                              addr_space="Shared").ap()
    cc2b_out = nc.dram_tensor("cc2b_out", [NC_N * (QLR // 2), SL], bf,
                              addr_space="Shared").ap()

    KH = HID // 128       # 40 k-chunks of the model dim
    KQ = QLR // 128       # 12 chunks of the q-lora dim
    KC = KVLR // 128      # 4 chunks of the kv-lora dim
    SC = S // 512         # 4 sequence chunks of 512
    SB = S // 128         # 16 sequence blocks of 128
    RPC = 512 // SL       # AG rank-blocks per 512-wide seq chunk

    with tile.TileContext(nc) as tc, ExitStack() as ctx:
        def pool(name, bufs, where=ctx):
            return where.enter_context(tc.tile_pool(name=name, bufs=bufs))

        # ---- persistent pools (live whole program) ----
        p_one = pool("ones", 2)
        p_wqb = pool("wqb", 6)
        p_wo = pool("wo", 4)
        p_cs = pool("cs", 4)
        p_K = pool("Kt", 4)
        p_V = pool("Vt", 16)
        p_kpeg = pool("kpeg", 4)
        p_sml = pool("sml", 3)
        p_qlg = pool("qlg", 12)
        p_Qn = pool("Qn", 5)
        p_pe = pool("pe", 4)
        p_f32 = pool("fr32", 2)
        p_rope = pool("rope", 4)
        p_P = pool("Pt", 3)
        p_acc = pool("acc", 2)
        p_oT = pool("oT", 5)
        p_bc = pool("bc", 3)
        p_msk = pool("msk", 1)
        p_out = pool("outst", 2)

        pp_mm = ctx.enter_context(
            tc.tile_pool(name="pmm", bufs=7, space="PSUM"))
        pp_sm = ctx.enter_context(
            tc.tile_pool(name="psm", bufs=1, space="PSUM"))

        ones_col = p_one.tile([128, 1], bf)       # lhsT for column sums
        nc.vector.memset(ones_col[:], 1.0)
        ones_f32 = p_one.tile([128, 1], f32, tag="onesf", name="onesf")
        nc.vector.memset(ones_f32[:], 1.0)
        eps_t = p_one.tile([1, 1], f32, tag="eps", name="eps")
        nc.vector.memset(eps_t[:], EPS)

        with ExitStack() as fctx:
            # ---- front-phase pools (released before attention) ----
            p_hs = pool("hs", 4, fctx)
            p_w = pool("wstr", 2, fctx)
            p_raw = pool("raw", 6, fctx)
            p_sq = pool("sqt", 2, fctx)
            p_scn = pool("scn", 3, fctx)
            p_csl = pool("csl", 2, fctx)
            p_ckvg = pool("ckvg", 16, fctx)
            p_kc = pool("kc", 4, fctx)
            p_vc = pool("vc", 1, fctx)

            # ------------- FRONT 1: ckv + k_pe pass (AG first) -------------
            # DMAs fetch two 128-row chunks per issue (3D access pattern)
            # to halve per-issue overhead on the DGE rings.
            W2 = KVLR + 128
            ckv_ps = [pp_mm.tile([128, SL], f32, tag="mm", name="mm")
                      for _ in range(KC)]
            pe_ps = pp_mm.tile([128, SL], f32, tag="mm", name="mm")
            for kp in range(KH // 2):
                hst = p_hs.tile([128, 2 * SL], bf, tag="hs", name="hs")
                nc.sync.dma_start(
                    hst[:, :].rearrange("p (b s) -> p b s", b=2),
                    hsT_d[kp * 256:(kp + 1) * 256, :]
                    .rearrange("(b p) s -> p b s", b=2))
                wkv = p_w.tile([128, 2 * W2], bf, tag="wst", name="wkv",
                               padded_shape=[128, 1536])
                nc.scalar.dma_start(
                    wkv[:, :].rearrange("p (b s) -> p b s", b=2),
                    wkvk_d[kp * 256:(kp + 1) * 256, :]
                    .rearrange("(b p) s -> p b s", b=2))
                for sub in range(2):
                    k = 2 * kp + sub
                    hs_s = hst[:, sub * SL:(sub + 1) * SL]
                    for c in range(KC):
                        nc.tensor.matmul(
                            ckv_ps[c][:],
                            wkv[:, sub * W2 + c * 128:sub * W2 + (c + 1) * 128],
                            hs_s, start=(k == 0), stop=(k == KH - 1))
                    nc.tensor.matmul(
                        pe_ps[:], wkv[:, sub * W2 + KVLR:sub * W2 + W2],
                        hs_s, start=(k == 0), stop=(k == KH - 1))
            ssq_kv = pp_sm.tile([1, SL], f32, tag="sm", name="sm")
            for c in range(KC):
                sq = p_sq.tile([128, SL], bf, tag="sq", name="sq")
                nc.scalar.activation(sq[:], ckv_ps[c][:], AF.Square)
                nc.tensor.matmul(ssq_kv[:], ones_col[:], sq[:],
                                 start=(c == 0), stop=(c == KC - 1))
            t_kv = p_sml.tile([1, SL], f32, tag="sml", name="sml")
            nc.scalar.activation(t_kv[:], ssq_kv[:], AF.Sqrt,
                                 bias=eps_t[:], scale=1.0 / KVLR)
            bkv = p_bc.tile([128, 512], f32, tag="bc", name="bc")
            nc.gpsimd.partition_broadcast(bkv[:, :SL], t_kv[:])
            rkv = p_bc.tile([128, 512], f32, tag="bc", name="bc")
            nc.vector.reciprocal(rkv[:, :SL], bkv[:, :SL])
            for c in range(KC):
                cn = p_scn.tile([128, SL], bf, tag="scn", name="scn")
                nc.vector.tensor_mul(cn[:], ckv_ps[c][:], rkv[:, :SL])
                nc.gpsimd.dma_start(cc1_in[c * 128:(c + 1) * 128, :], cn[:])
            # k_pe rope: rot = partition swap of pe (sign folded into sinl)
            cosl_t = p_csl.tile([128, SL], bf, tag="csl", name="csl")
            sinl_t = p_csl.tile([128, SL], bf, tag="csl", name="csl")
            nc.sync.dma_start(cosl_t[:], cosl_d[:, :])
            nc.sync.dma_start(sinl_t[:], sinl_d[:, :])
            pe_sb = p_scn.tile([128, SL], bf, tag="scn", name="scn")
            nc.scalar.activation(pe_sb[:], pe_ps[:], AF.Copy)
            rot_sb = p_scn.tile([128, SL], bf, tag="scn", name="scn")
            for h in range(4):
                src = (h ^ 1) * 32
                nc.gpsimd.dma_start(rot_sb[h * 32:(h + 1) * 32, :],
                                    pe_sb[src:src + 32, :])
            t1 = p_f32.tile([128, SL], f32, tag="f32", name="f32")
            t2 = p_f32.tile([128, SL], f32, tag="f32", name="f32")
            nc.vector.tensor_mul(t1[:], pe_ps[:], cosl_t[:])
            nc.vector.tensor_mul(t2[:], rot_sb[:], sinl_t[:])
            kpe_n = p_scn.tile([128, SL], bf, tag="scn", name="scn")
            nc.vector.tensor_add(kpe_n[:], t1[:], t2[:])
            nc.gpsimd.dma_start(cc1_in[KVLR:KVLR + 128, :], kpe_n[:])

            nc.gpsimd.collective_compute(
                "AllGather", mybir.AluOpType.bypass,
                ins=[cc1_in[:]], outs=[cc1_out[:]],
                replica_groups=[list(range(NC_N))],
            )

            # ------------- FRONT 2: q_lora pass -------------
            ssq_q = pp_sm.tile([1, SL], f32, tag="sm", name="sm")
            raw_q = []
            g1_ps = None
            for g in range(2):
                ql_ps = [pp_mm.tile([128, SL], f32, tag="mm", name="mm")
                         for _ in range(KQ // 2)]
                for kp in range(KH // 2):
                    hst = p_hs.tile([128, 2 * SL], bf, tag="hs", name="hs")
                    nc.gpsimd.dma_start(
                        hst[:, :].rearrange("p (b s) -> p b s", b=2),
                        hsT_d[kp * 256:(kp + 1) * 256, :]
                        .rearrange("(b p) s -> p b s", b=2))
                    w = p_w.tile([128, QLR], bf, tag="wst", name="wqa")
                    nc.sync.dma_start(
                        w[:, :].rearrange("p (b s) -> p b s", b=2),
                        wqa_d[kp * 256:(kp + 1) * 256,
                              g * (QLR // 2):(g + 1) * (QLR // 2)]
                        .rearrange("(b p) s -> p b s", b=2))
                    for sub in range(2):
                        k = 2 * kp + sub
                        hs_s = hst[:, sub * SL:(sub + 1) * SL]
                        for mi in range(KQ // 2):
                            nc.tensor.matmul(
                                ql_ps[mi][:],
                                w[:, sub * 768 + mi * 128:
                                  sub * 768 + (mi + 1) * 128],
                                hs_s, start=(k == 0), stop=(k == KH - 1))
                for mi in range(KQ // 2):
                    m = g * (KQ // 2) + mi
                    sq = p_sq.tile([128, SL], bf, tag="sq", name="sq")
                    nc.scalar.activation(sq[:], ql_ps[mi][:], AF.Square)
                    nc.tensor.matmul(ssq_q[:], ones_col[:], sq[:],
                                     start=(m == 0), stop=(m == KQ - 1))
                    if g == 0:
                        r = p_raw.tile([128, SL], bf, tag="raw", name="raw")
                        nc.scalar.activation(r[:], ql_ps[mi][:], AF.Copy)
                        raw_q.append(r)
                if g == 1:
                    g1_ps = ql_ps
            t_q = p_sml.tile([1, SL], f32, tag="sml", name="sml")
            nc.scalar.activation(t_q[:], ssq_q[:], AF.Sqrt,
                                 bias=eps_t[:], scale=1.0 / QLR)
            bq = p_bc.tile([128, 512], f32, tag="bc", name="bc")
            nc.gpsimd.partition_broadcast(bq[:, :SL], t_q[:])
            rq = p_bc.tile([128, 512], f32, tag="bc", name="bc")
            nc.vector.reciprocal(rq[:, :SL], bq[:, :SL])
            for m in range(KQ):
                qn = p_scn.tile([128, SL], bf, tag="scn", name="scn")
                src = raw_q[m][:] if m < KQ // 2 else g1_ps[m - KQ // 2][:]
                nc.vector.tensor_mul(qn[:], src, rq[:, :SL])
                nc.gpsimd.dma_start(cc2_in[m * 128:(m + 1) * 128, :], qn[:])
                if m == KQ // 2 - 1:
                    nc.gpsimd.collective_compute(
                        "AllGather", mybir.AluOpType.bypass,
                        ins=[cc2_in[0:QLR // 2, :]], outs=[cc2a_out[:]],
                        replica_groups=[list(range(NC_N))],
                    )
            nc.gpsimd.collective_compute(
                "AllGather", mybir.AluOpType.bypass,
                ins=[cc2_in[QLR // 2:QLR, :]], outs=[cc2b_out[:]],
                replica_groups=[list(range(NC_N))],
            )

            # resident back-end weights: prefetch on sync while AGs fly
            wqb6 = []
            for kp in range(KQ // 2):
                t = p_wqb.tile([128, 1536], bf, tag="wqb", name="wqb")
                nc.sync.dma_start(
                    t[:, :].rearrange("p (b s) -> p b s", b=2),
                    wqb_d[kp * 256:(kp + 1) * 256, :]
                    .rearrange("(b p) s -> p b s", b=2))
                wqb6.append(t)

            def wqb_s(k, off, w):
                return wqb6[k // 2][:, (k % 2) * 768 + off:
                                    (k % 2) * 768 + off + w]
            kc4 = []
            for i in range(HPC):
                t = p_kc.tile([128, KC * DN], bf, tag="kc", name="kc")
                nc.sync.dma_start(
                    t[:, :].rearrange("p (b s) -> p b s", b=KC),
                    kct_d[i * KVLR:(i + 1) * KVLR, :]
                    .rearrange("(b p) s -> p b s", b=KC))
                kc4.append(t)
            vc4 = p_vc.tile([128, KC * HPC * DV], bf, tag="vc", name="vc")
            nc.sync.dma_start(
                vc4[:, :].rearrange("p (b s) -> p b s", b=KC),
                vcp_d[:, :].rearrange("(b p) s -> p b s", b=KC))
            tri_t = p_msk.tile([128, 128], bf, tag="msk", name="msk")
            nc.sync.dma_start(tri_t[:], tri_d[:, :])
            wo_t = []
            for i in range(HPC):
                t = p_wo.tile([128, HID], bf, tag="wo", name="wo")
                nc.sync.dma_start(t[:], wo_d[i * DV:(i + 1) * DV, :])
                wo_t.append(t)

            # gathered ckv / kpe (scalar ring, waits on AG1)
            ckvg = {}
            for c in range(KC):
                for sc in range(SC):
                    t = p_ckvg.tile([128, 512], bf, tag="ckvg", name="ckvg")
                    for half in range(RPC):
                        r = RPC * sc + half
                        nc.scalar.dma_start(
                            t[:, half * SL:(half + 1) * SL],
                            cc1_out[r * (KVLR + 128) + c * 128:
                                    r * (KVLR + 128) + (c + 1) * 128, :])
                    ckvg[(c, sc)] = t
            kpeg = {}
            for sc in range(SC):
                t = p_kpeg.tile([128, 512], bf, tag="kpeg", name="kpeg")
                for half in range(RPC):
                    r = RPC * sc + half
                    nc.scalar.dma_start(
                        t[:, half * SL:(half + 1) * SL],
                        cc1_out[r * (KVLR + 128) + KVLR:
                                r * (KVLR + 128) + KVLR + 128, :])
                kpeg[sc] = t

            # K^T per head: [DN, S] — overlaps the q AllGathers
            K_t = []
            for i in range(HPC):
                kt = p_K.tile([128, S], bf, tag="K", name="K")
                K_t.append(kt)
                for sc in range(SC):
                    ps = pp_mm.tile([128, 512], f32, tag="mm", name="mm")
                    for c in range(KC):
                        nc.tensor.matmul(ps[:],
                                         kc4[i][:, c * DN:(c + 1) * DN],
                                         ckvg[(c, sc)][:],
                                         start=(c == 0), stop=(c == KC - 1))
                    nc.scalar.activation(kt[:, sc * 512:(sc + 1) * 512],
                                         ps[:], AF.Copy)

            # V natural: per seq-block [128, 4*DV]
            V_t = []
            for sb in range(SB):
                ps = pp_mm.tile([128, 512], f32, tag="mm", name="mm")
                for c in range(KC):
                    nc.tensor.matmul(
                        ps[:],
                        ckvg[(c, sb // 4)][:, (sb % 4) * 128:(sb % 4 + 1) * 128],
                        vc4[:, c * 512:(c + 1) * 512],
                        start=(c == 0), stop=(c == KC - 1))
                vt = p_V.tile([128, HPC * DV], bf, tag="V", name="V")
                nc.scalar.activation(vt[:], ps[:], AF.Copy)
                V_t.append(vt)
        # ---- front pools released here ----

        # ---------------- BACK: head-sharded attention ---------------------
        for sc in range(SC):
            # gathered q_lora^T tiles for this seq chunk (a-half first)
            qlg = []
            for k in range(KQ):
                t = p_qlg.tile([128, 512], bf, tag="qlg", name="qlg")
                buf = cc2a_out if k < KQ // 2 else cc2b_out
                kk = k if k < KQ // 2 else k - KQ // 2
                for half in range(RPC):
                    r = RPC * sc + half
                    nc.sync.dma_start(
                        t[:, half * SL:(half + 1) * SL],
                        buf[r * (QLR // 2) + kk * 128:
                            r * (QLR // 2) + (kk + 1) * 128, :])
                qlg.append(t)
            cosf_t = p_cs.tile([128, 512], bf, tag="cs", name="cs")
            sinf_t = p_cs.tile([128, 512], bf, tag="cs", name="cs")
            nc.sync.dma_start(cosf_t[:], cosf_d[:, sc * 512:(sc + 1) * 512])
            nc.sync.dma_start(sinf_t[:], sinf_d[:, sc * 512:(sc + 1) * 512])
            # Q^T nope per head (transient)
            qn_t = []
            for i in range(HPC):
                ps = pp_mm.tile([128, 512], f32, tag="mm", name="mm")
                for k in range(KQ):
                    nc.tensor.matmul(ps[:], wqb_s(k, i * 128, 128),
                                     qlg[k][:], start=(k == 0),
                                     stop=(k == KQ - 1))
                qt = p_Qn.tile([128, 512], bf, tag="Qn", name="Qn")
                nc.scalar.activation(qt[:], ps[:], AF.Copy)
                qn_t.append(qt)
            # Q^T pe packs + rope (rot = partition swap, sign in sinf)
            roped = []
            for pkt in range(2):
                ps_pe = pp_mm.tile([128, 512], f32, tag="mm", name="mm")
                for k in range(KQ):
                    nc.tensor.matmul(
                        ps_pe[:], wqb_s(k, 512 + pkt * 128, 128),
                        qlg[k][:], start=(k == 0), stop=(k == KQ - 1))
                qpe_sb = p_pe.tile([128, 512], bf, tag="pe", name="pe")
                nc.scalar.activation(qpe_sb[:], ps_pe[:], AF.Copy)
                qrot_sb = p_pe.tile([128, 512], bf, tag="pe", name="pe")
                for h in range(4):
                    src = (h ^ 1) * 32
                    nc.gpsimd.dma_start(qrot_sb[h * 32:(h + 1) * 32, :],
                                        qpe_sb[src:src + 32, :])
                u1 = p_f32.tile([128, 512], f32, tag="rope32", name="rope32")
                u2 = p_f32.tile([128, 512], f32, tag="rope32", name="rope32")
                nc.vector.tensor_mul(u1[:], ps_pe[:], cosf_t[:])
                nc.vector.tensor_mul(u2[:], qrot_sb[:], sinf_t[:])
                rp = p_rope.tile([128, 512], bf, tag="rope", name="rope")
                nc.vector.tensor_add(rp[:], u1[:], u2[:])
                roped.append(rp)

            # attention for each head on this seq chunk
            oT = {}
            for i in range(HPC):
                pkt, hp = i // 2, i % 2
                o_ps = pp_mm.tile([128, 512], f32, tag="mm", name="mm")
                acc = p_acc.tile([128, 512], f32, tag="acc", name="acc")
                nj = 4 * sc + 4
                for j in range(nj):
                    lo = (j - 4 * sc) * 128 if j >= 4 * sc else 0
                    s_ps = pp_mm.tile([128, 512], f32, tag="mm", name="mm")
                    nc.tensor.matmul(s_ps[:, lo:512],
                                     K_t[i][:, j * 128:(j + 1) * 128],
                                     qn_t[i][:, lo:512],
                                     start=True, stop=False)
                    nc.tensor.matmul(
                        s_ps[:, lo:512],
                        kpeg[j // 4][hp * 64:(hp + 1) * 64,
                                     (j % 4) * 128:(j % 4 + 1) * 128],
                        roped[pkt][hp * 64:(hp + 1) * 64, lo:512],
                        start=False, stop=True)
                    if j >= 4 * sc:
                        nc.vector.tensor_add(s_ps[:, lo:lo + 128],
                                             s_ps[:, lo:lo + 128], tri_t[:])
                    pt = p_P.tile([128, 512], bf, tag="P", name="P")
                    nc.scalar.activation(pt[:, lo:512], s_ps[:, lo:512],
                                         AF.Exp)
                    if j == 0:
                        nc.vector.tensor_copy(acc[:], pt[:])
                    else:
                        nc.vector.tensor_add(acc[:, lo:512], acc[:, lo:512],
                                             pt[:, lo:512])
                    nc.tensor.matmul(o_ps[:, lo:512],
                                     V_t[j][:, i * DV:(i + 1) * DV],
                                     pt[:, lo:512],
                                     start=(j == 0), stop=(j == nj - 1),
                                     skip_group_check=True)
                d_ps = pp_sm.tile([1, 512], f32, tag="sm", name="sm")
                nc.tensor.matmul(d_ps[:], ones_f32[:], acc[:],
                                 start=True, stop=True)
                d_sb = p_sml.tile([1, 512], f32, tag="sml", name="sml")
                nc.scalar.activation(d_sb[:], d_ps[:], AF.Copy)
                dbc = p_bc.tile([128, 512], f32, tag="bc", name="bc")
                nc.gpsimd.partition_broadcast(dbc[:], d_sb[:])
                dinv = p_bc.tile([128, 512], f32, tag="bc", name="bc")
                nc.vector.reciprocal(dinv[:], dbc[:])
                ot = p_oT.tile([128, 512], bf, tag="oT", name="oT")
                nc.vector.tensor_mul(ot[:], o_ps[:], dinv[:])
                oT[i] = ot

            # w_o partial for this seq chunk (overlaps later chunks)
            for sbl in range(4):
                sb = sc * 4 + sbl
                stage = p_out.tile([128, HID // 2], bf, tag="outst",
                                   name="outst")
                for n in range(HID // 512):
                    ps = pp_mm.tile([128, 512], f32, tag="mm", name="mm")
                    for i in range(HPC):
                        nc.tensor.matmul(
                            ps[:], oT[i][:, sbl * 128:(sbl + 1) * 128],
                            wo_t[i][:, n * 512:(n + 1) * 512],
                            start=(i == 0), stop=(i == HPC - 1))
                    half, off = n // 5, (n % 5) * 512
                    if n % 2 == 0:
                        nc.vector.tensor_copy(stage[:, off:off + 512], ps[:])
                    else:
                        nc.scalar.activation(stage[:, off:off + 512], ps[:],
                                             AF.Copy)
                    if n % 5 == 4:
                        nc.gpsimd.dma_start(
                            out_d[sb * 128:(sb + 1) * 128,
                                  half * 2560:(half + 1) * 2560],
                            stage[:, :])
                        if n == 4:
                            stage = p_out.tile([128, HID // 2], bf,
                                               tag="outst", name="outst")

    nc.compile()
    return nc


def _prep_inputs(inputs):
    """Host-side sharding + weight folding. Returns in_maps (list of 8 dicts)."""
    BF = _bf16()

    hs = np.asarray(inputs['hidden_states'], np.float32)
    pos = np.asarray(inputs['positions'])
    w_qa = np.asarray(inputs['w_qa'], np.float32)
    q_a_ln_w = np.asarray(inputs['q_a_ln_w'], np.float32)
    w_qb = np.asarray(inputs['w_qb'], np.float32)
    w_kva = np.asarray(inputs['w_kva'], np.float32)
    kv_a_ln_w = np.asarray(inputs['kv_a_ln_w'], np.float32)
    kc = np.asarray(inputs['kc'], np.float32)
    vc = np.asarray(inputs['vc'], np.float32)
    w_o = np.asarray(inputs['w_o'], np.float32)

    perm = np.concatenate([np.arange(0, DR, 2), np.arange(1, DR, 2)])
    inv_freq = 1.0 / (ROPE_BASE ** (np.arange(0, DR, 2, dtype=np.float64) / DR))
    freqs = pos.astype(np.float64)[None, :] * inv_freq[:, None]     # [32, S]
    cosT = np.cos(freqs).astype(np.float32)
    sinT = np.sin(freqs).astype(np.float32)
    cos128 = np.tile(cosT, (4, 1)).astype(BF)                        # [128, S]
    # rot rows carry the swapped halves; the rotation sign lives here:
    # rows 0-31 multiply -sin, rows 32-63 multiply +sin (per 64-block).
    sin128s = np.tile(np.concatenate([-sinT, sinT], 0), (2, 1)).astype(BF)

    scale = DQ ** -0.5
    w_qb_eff = ((w_qb * q_a_ln_w[:, None]) * scale).reshape(QLR, H, DQ)

    w_pe = w_kva[:, KVLR:][:, perm]
    wkvk = np.concatenate([w_kva[:, :KVLR], w_pe, w_pe], 1).astype(BF)

    kc_f = kc * kv_a_ln_w[None, None, :]
    vc_f = vc * kv_a_ln_w[None, :, None]

    kr = np.arange(128)[:, None]
    qc = np.arange(128)[None, :]
    tri = np.where(kr > qc, -1e30, 0.0).astype(np.float32).astype(BF)

    wqa_b = w_qa.astype(BF)

    in_maps = []
    for core in range(NC_N):
        rows = slice(core * SL, (core + 1) * SL)
        h0 = core * HPC

        wqb_all = np.empty((QLR, 768), np.float32)
        for i in range(HPC):
            wqb_all[:, i * 128:(i + 1) * 128] = w_qb_eff[:, h0 + i, :DN]
        for pkt in range(2):
            a, b = h0 + 2 * pkt, h0 + 2 * pkt + 1
            pe_a = w_qb_eff[:, a, DN:][:, perm]
            pe_b = w_qb_eff[:, b, DN:][:, perm]
            wqb_all[:, 512 + pkt * 128:512 + pkt * 128 + 64] = pe_a
            wqb_all[:, 512 + pkt * 128 + 64:512 + (pkt + 1) * 128] = pe_b

        kct = np.concatenate([kc_f[h0 + i].T for i in range(HPC)], 0)
        vcp = np.concatenate([vc_f[h0 + i] for i in range(HPC)], 1)
        wo_sh = w_o[h0 * DV:(h0 + HPC) * DV, :]

        in_maps.append({
            "hsT": np.ascontiguousarray(hs[rows].T).astype(BF),
            "wqa": wqa_b,
            "wkvk": wkvk,
            "cosl": np.ascontiguousarray(cos128[:, rows]),
            "sinl": np.ascontiguousarray(sin128s[:, rows]),
            "cosf": cos128,
            "sinf": sin128s,
            "wqb": wqb_all.astype(BF),
            "kct": kct.astype(BF),
            "vcp": vcp.astype(BF),
            "wo": wo_sh.astype(BF),
            "tri": tri,
        })
    return in_maps


def _get_program():
    if "nc" not in _CACHE:
        _CACHE["nc"] = _build_program()
    return _CACHE["nc"]


def run(inputs, trace=False, trace_kwargs=None):
    """Build (cached), run on 8 cores, return (output, BassKernelResults)."""
    from concourse.bass_utils import run_bass_kernel_spmd

    nc = _get_program()
    in_maps = _prep_inputs(inputs)
    res = run_bass_kernel_spmd(nc, in_maps, list(range(NC_N)),
                               trace=trace, **(trace_kwargs or {}))
    out = np.zeros((S, HID), np.float32)
    for r in res.results:
        out += r["out_partial"].astype(np.float32)
    return out, res


def kernel(**inputs) -> np.ndarray:
    out, _ = run(inputs, trace=False)
    return out


# revision 27
# speedup vs baseline: 1.1989x; 1.0090x over previous
"""Trainium2 Bass kernel for a DeepseekV2 decoder-layer attention block
(MLA prefill, fp32 reference) distributed across 8 NeuronCores.

Strategy (single NEFF, SPMD on 8 cores):
  - Sequence-shard the shared projections: each core computes ckv / k_pe
    (RMS-normed / roped) then q_lora for its 256 rows of the sequence, in
    transposed layout; on-device AllGathers replicate them (ckv AG first so
    the K/V expansion overlaps the q AllGathers).
  - Head-shard the rest (4 heads per core): q_b projection + RoPE, kc/vc
    expansion, causal attention (scores computed transposed so the attn@v
    matmul needs no transposes), and a row-shard of w_o.
  - Each core emits a partial [S, HID] bf16 output; the host sums the 8
    partials (the output all-reduce) in fp32.

Tensor-engine economies vs the naive formulation:
  - RoPE "rotate-half" is a partition permutation: done with 4 small
    SBUF->SBUF DMA row swaps + sign-folded sin tiles instead of duplicate
    sign-flipped weight-matmul chains.
  - softmax denominators accumulate on the vector engine (exp tiles summed
    across key blocks); one [1,512] ones-matmul per (head, chunk) finishes
    the partition reduction.
  - causal diagonal blocks compute only the unmasked column range; the
    triangle mask is applied additively (-1e30) on PSUM before exp.
  - reciprocals run after a partition-broadcast so all 128 vector lanes
    work; w_o stays SBUF-resident; outputs staged/written as bf16.
"""

import numpy as np

S, HID, H = 2048, 5120, 32
QLR, KVLR = 1536, 512
DN, DR, DV = 128, 64, 128
DQ = DN + DR
NC_N = 8
HPC = H // NC_N          # heads per core
SL = S // NC_N           # sequence rows per core (front end)
ROPE_BASE, EPS = 10000.0, 1e-6

_CACHE = {}


def _bf16():
    import ml_dtypes
    return np.dtype(ml_dtypes.bfloat16)


def _build_program():
    import concourse.bass as bass
    import concourse.tile as tile
    from concourse import bacc, mybir
    from contextlib import ExitStack

    f32 = mybir.dt.float32
    bf = mybir.dt.bfloat16
    AF = mybir.ActivationFunctionType

    nc = bacc.Bacc("TRN2", target_bir_lowering=False, debug=False,
                   num_devices=NC_N)

    def din(name, shape, dt=bf):
        return nc.dram_tensor(name, list(shape), dt, kind="ExternalInput").ap()

    hsT_d = din("hsT", (HID, SL))
    wqa_d = din("wqa", (HID, QLR))
    wkvk_d = din("wkvk", (HID, KVLR + 128))   # ckv cols | pe-dup cols
    cosl_d = din("cosl", (128, SL))
    sinl_d = din("sinl", (128, SL))           # sign-folded
    cosf_d = din("cosf", (128, S))
    sinf_d = din("sinf", (128, S))            # sign-folded
    wqb_d = din("wqb", (QLR, 768))            # nope(4x128) | pe(2x128)
    kct_d = din("kct", (HPC * KVLR, DN))      # per head: kc'^T [KVLR, DN]
    vcp_d = din("vcp", (KVLR, HPC * DV))
    wo_d = din("wo", (HPC * DV, HID))
    tri_d = din("tri", (128, 128))            # additive causal mask (scores^T)
    out_d = nc.dram_tensor("out_partial", [S, HID], bf,
                           kind="ExternalOutput").ap()

    cc1_in = nc.dram_tensor("cc1_in", [KVLR + 128, SL], bf).ap()
    cc1_out = nc.dram_tensor("cc1_out", [NC_N * (KVLR + 128), SL], bf,
                             addr_space="Shared").ap()
    cc2_in = nc.dram_tensor("cc2_in", [QLR, SL], bf).ap()
    cc2a_out = nc.dram_tensor("cc2a_out", [NC_N * (QLR // 2), SL], bf,
                              addr_space="Shared").ap()
    cc2b_out = nc.dram_tensor("cc2b_out", [NC_N * (QLR // 2), SL], bf,
                              addr_space="Shared").ap()

    KH = HID // 128       # 40 k-chunks of the model dim
    KQ = QLR // 128       # 12 chunks of the q-lora dim
    KC = KVLR // 128      # 4 chunks of the kv-lora dim
    SC = S // 512         # 4 sequence chunks of 512
    SB = S // 128         # 16 sequence blocks of 128
    RPC = 512 // SL       # AG rank-blocks per 512-wide seq chunk

    with tile.TileContext(nc) as tc, ExitStack() as ctx:
        def pool(name, bufs, where=ctx):
            return where.enter_context(tc.tile_pool(name=name, bufs=bufs))

        # ---- persistent pools (live whole program) ----
        p_one = pool("ones", 2)
        p_wqb = pool("wqb", 6)
        p_wo = pool("wo", 4)
        p_cs = pool("cs", 4)
        p_K = pool("Kt", 4)
        p_V = pool("Vt", 16)
        p_kpeg = pool("kpeg", 4)
        p_sml = pool("sml", 3)
        p_qlg = pool("qlg", 12)
        p_Qn = pool("Qn", 5)
        p_pe = pool("pe", 4)
        p_f32 = pool("fr32", 2)
        p_rope = pool("rope", 4)
        p_P = pool("Pt", 3)
        p_acc = pool("acc", 2)
        p_oT = pool("oT", 5)
        p_bc = pool("bc", 3)
        p_msk = pool("msk", 1)
        p_out = pool("outst", 2)

        pp_mm = ctx.enter_context(
            tc.tile_pool(name="pmm", bufs=7, space="PSUM"))
        pp_sm = ctx.enter_context(
            tc.tile_pool(name="psm", bufs=1, space="PSUM"))

        ones_col = p_one.tile([128, 1], bf)       # lhsT for column sums
        nc.vector.memset(ones_col[:], 1.0)
        ones_f32 = p_one.tile([128, 1], f32, tag="onesf", name="onesf")
        nc.vector.memset(ones_f32[:], 1.0)
        eps_t = p_one.tile([1, 1], f32, tag="eps", name="eps")
        nc.vector.memset(eps_t[:], EPS)

        with ExitStack() as fctx:
            # ---- front-phase pools (released before attention) ----
            p_hs = pool("hs", 4, fctx)
            p_w = pool("wstr", 2, fctx)
            p_raw = pool("raw", 6, fctx)
            p_sq = pool("sqt", 2, fctx)
            p_scn = pool("scn", 3, fctx)
            p_csl = pool("csl", 2, fctx)
            p_ckvg = pool("ckvg", 16, fctx)
            p_kc = pool("kc", 4, fctx)
            p_vc = pool("vc", 1, fctx)

            # ------------- FRONT 1: ckv + k_pe pass (AG first) -------------
            # DMAs fetch two 128-row chunks per issue (3D access pattern)
            # to halve per-issue overhead on the DGE rings.
            W2 = KVLR + 128
            ckv_ps = [pp_mm.tile([128, SL], f32, tag="mm", name="mm")
                      for _ in range(KC)]
            pe_ps = pp_mm.tile([128, SL], f32, tag="mm", name="mm")
            for kp in range(KH // 2):
                hst = p_hs.tile([128, 2 * SL], bf, tag="hs", name="hs")
                nc.sync.dma_start(
                    hst[:, :].rearrange("p (b s) -> p b s", b=2),
                    hsT_d[kp * 256:(kp + 1) * 256, :]
                    .rearrange("(b p) s -> p b s", b=2))
                wkv = p_w.tile([128, 2 * W2], bf, tag="wst", name="wkv",
                               padded_shape=[128, 1536])
                nc.scalar.dma_start(
                    wkv[:, :].rearrange("p (b s) -> p b s", b=2),
                    wkvk_d[kp * 256:(kp + 1) * 256, :]
                    .rearrange("(b p) s -> p b s", b=2))
                for sub in range(2):
                    k = 2 * kp + sub
                    hs_s = hst[:, sub * SL:(sub + 1) * SL]
                    for c in range(KC):
                        nc.tensor.matmul(
                            ckv_ps[c][:],
                            wkv[:, sub * W2 + c * 128:sub * W2 + (c + 1) * 128],
                            hs_s, start=(k == 0), stop=(k == KH - 1))
                    nc.tensor.matmul(
                        pe_ps[:], wkv[:, sub * W2 + KVLR:sub * W2 + W2],
                        hs_s, start=(k == 0), stop=(k == KH - 1))
            ssq_kv = pp_sm.tile([1, SL], f32, tag="sm", name="sm")
            for c in range(KC):
                sq = p_sq.tile([128, SL], bf, tag="sq", name="sq")
                nc.scalar.activation(sq[:], ckv_ps[c][:], AF.Square)
                nc.tensor.matmul(ssq_kv[:], ones_col[:], sq[:],
                                 start=(c == 0), stop=(c == KC - 1))
            t_kv = p_sml.tile([1, SL], f32, tag="sml", name="sml")
            nc.scalar.activation(t_kv[:], ssq_kv[:], AF.Sqrt,
                                 bias=eps_t[:], scale=1.0 / KVLR)
            bkv = p_bc.tile([128, 512], f32, tag="bc", name="bc")
            nc.gpsimd.partition_broadcast(bkv[:, :SL], t_kv[:])
            rkv = p_bc.tile([128, 512], f32, tag="bc", name="bc")
            nc.vector.reciprocal(rkv[:, :SL], bkv[:, :SL])
            for c in range(KC):
                cn = p_scn.tile([128, SL], bf, tag="scn", name="scn")
                nc.vector.tensor_mul(cn[:], ckv_ps[c][:], rkv[:, :SL])
                nc.gpsimd.dma_start(cc1_in[c * 128:(c + 1) * 128, :], cn[:])
            # k_pe rope: rot = partition swap of pe (sign folded into sinl)
            cosl_t = p_csl.tile([128, SL], bf, tag="csl", name="csl")
            sinl_t = p_csl.tile([128, SL], bf, tag="csl", name="csl")
            nc.sync.dma_start(cosl_t[:], cosl_d[:, :])
            nc.sync.dma_start(sinl_t[:], sinl_d[:, :])
            pe_sb = p_scn.tile([128, SL], bf, tag="scn", name="scn")
            nc.scalar.activation(pe_sb[:], pe_ps[:], AF.Copy)
            rot_sb = p_scn.tile([128, SL], bf, tag="scn", name="scn")
            for h in range(4):
                src = (h ^ 1) * 32
                nc.gpsimd.dma_start(rot_sb[h * 32:(h + 1) * 32, :],
                                    pe_sb[src:src + 32, :])
            t1 = p_f32.tile([128, SL], f32, tag="f32", name="f32")
            t2 = p_f32.tile([128, SL], f32, tag="f32", name="f32")
            nc.vector.tensor_mul(t1[:], pe_ps[:], cosl_t[:])
            nc.vector.tensor_mul(t2[:], rot_sb[:], sinl_t[:])
            kpe_n = p_scn.tile([128, SL], bf, tag="scn", name="scn")
            nc.vector.tensor_add(kpe_n[:], t1[:], t2[:])
            nc.gpsimd.dma_start(cc1_in[KVLR:KVLR + 128, :], kpe_n[:])

            nc.gpsimd.collective_compute(
                "AllGather", mybir.AluOpType.bypass,
                ins=[cc1_in[:]], outs=[cc1_out[:]],
                replica_groups=[list(range(NC_N))],
            )

            # gathered ckv / kpe: on the gpsimd ring right behind AG1, so
            # they land while the q pass still owns tensor/sync/scalar.
            ckvg = {}
            for c in range(KC):
                for sc in range(SC):
                    t = p_ckvg.tile([128, 512], bf, tag="ckvg", name="ckvg")
                    for half in range(RPC):
                        r = RPC * sc + half
                        nc.gpsimd.dma_start(
                            t[:, half * SL:(half + 1) * SL],
                            cc1_out[r * (KVLR + 128) + c * 128:
                                    r * (KVLR + 128) + (c + 1) * 128, :])
                    ckvg[(c, sc)] = t
            kpeg = {}
            for sc in range(SC):
                t = p_kpeg.tile([128, 512], bf, tag="kpeg", name="kpeg")
                for half in range(RPC):
                    r = RPC * sc + half
                    nc.gpsimd.dma_start(
                        t[:, half * SL:(half + 1) * SL],
                        cc1_out[r * (KVLR + 128) + KVLR:
                                r * (KVLR + 128) + KVLR + 128, :])
                kpeg[sc] = t

            # resident back-end weights: prefetch on sync before the q
            # stream so they arrive well ahead of the K/V expansion.
            wqb6 = []
            for kp in range(KQ // 2):
                t = p_wqb.tile([128, 1536], bf, tag="wqb", name="wqb")
                nc.sync.dma_start(
                    t[:, :].rearrange("p (b s) -> p b s", b=2),
                    wqb_d[kp * 256:(kp + 1) * 256, :]
                    .rearrange("(b p) s -> p b s", b=2))
                wqb6.append(t)

            def wqb_s(k, off, w):
                return wqb6[k // 2][:, (k % 2) * 768 + off:
                                    (k % 2) * 768 + off + w]
            kc4 = []
            for i in range(HPC):
                t = p_kc.tile([128, KC * DN], bf, tag="kc", name="kc")
                nc.sync.dma_start(
                    t[:, :].rearrange("p (b s) -> p b s", b=KC),
                    kct_d[i * KVLR:(i + 1) * KVLR, :]
                    .rearrange("(b p) s -> p b s", b=KC))
                kc4.append(t)
            vc4 = p_vc.tile([128, KC * HPC * DV], bf, tag="vc", name="vc")
            nc.sync.dma_start(
                vc4[:, :].rearrange("p (b s) -> p b s", b=KC),
                vcp_d[:, :].rearrange("(b p) s -> p b s", b=KC))
            tri_t = p_msk.tile([128, 128], bf, tag="msk", name="msk")
            nc.sync.dma_start(tri_t[:], tri_d[:, :])
            wo_t = []
            for i in range(HPC):
                t = p_wo.tile([128, HID], bf, tag="wo", name="wo")
                nc.sync.dma_start(t[:], wo_d[i * DV:(i + 1) * DV, :])
                wo_t.append(t)

            # ------------- FRONT 2: q_lora pass -------------
            ssq_q = pp_sm.tile([1, SL], f32, tag="sm", name="sm")
            raw_q = []
            g1_ps = None
            for g in range(2):
                ql_ps = [pp_mm.tile([128, SL], f32, tag="mm", name="mm")
                         for _ in range(KQ // 2)]
                for kp in range(KH // 2):
                    hst = p_hs.tile([128, 2 * SL], bf, tag="hs", name="hs")
                    nc.sync.dma_start(
                        hst[:, :].rearrange("p (b s) -> p b s", b=2),
                        hsT_d[kp * 256:(kp + 1) * 256, :]
                        .rearrange("(b p) s -> p b s", b=2))
                    w = p_w.tile([128, QLR], bf, tag="wst", name="wqa")
                    nc.scalar.dma_start(
                        w[:, :].rearrange("p (b s) -> p b s", b=2),
                        wqa_d[kp * 256:(kp + 1) * 256,
                              g * (QLR // 2):(g + 1) * (QLR // 2)]
                        .rearrange("(b p) s -> p b s", b=2))
                    for sub in range(2):
                        k = 2 * kp + sub
                        hs_s = hst[:, sub * SL:(sub + 1) * SL]
                        for mi in range(KQ // 2):
                            nc.tensor.matmul(
                                ql_ps[mi][:],
                                w[:, sub * 768 + mi * 128:
                                  sub * 768 + (mi + 1) * 128],
                                hs_s, start=(k == 0), stop=(k == KH - 1))
                for mi in range(KQ // 2):
                    m = g * (KQ // 2) + mi
                    sq = p_sq.tile([128, SL], bf, tag="sq", name="sq")
                    nc.scalar.activation(sq[:], ql_ps[mi][:], AF.Square)
                    nc.tensor.matmul(ssq_q[:], ones_col[:], sq[:],
                                     start=(m == 0), stop=(m == KQ - 1))
                    if g == 0:
                        r = p_raw.tile([128, SL], bf, tag="raw", name="raw")
                        nc.scalar.activation(r[:], ql_ps[mi][:], AF.Copy)
                        raw_q.append(r)
                if g == 1:
                    g1_ps = ql_ps
            t_q = p_sml.tile([1, SL], f32, tag="sml", name="sml")
            nc.scalar.activation(t_q[:], ssq_q[:], AF.Sqrt,
                                 bias=eps_t[:], scale=1.0 / QLR)
            bq = p_bc.tile([128, 512], f32, tag="bc", name="bc")
            nc.gpsimd.partition_broadcast(bq[:, :SL], t_q[:])
            rq = p_bc.tile([128, 512], f32, tag="bc", name="bc")
            nc.vector.reciprocal(rq[:, :SL], bq[:, :SL])
            for m in range(KQ):
                qn = p_scn.tile([128, SL], bf, tag="scn", name="scn")
                src = raw_q[m][:] if m < KQ // 2 else g1_ps[m - KQ // 2][:]
                nc.vector.tensor_mul(qn[:], src, rq[:, :SL])
                nc.gpsimd.dma_start(cc2_in[m * 128:(m + 1) * 128, :], qn[:])
                if m == KQ // 2 - 1:
                    nc.gpsimd.collective_compute(
                        "AllGather", mybir.AluOpType.bypass,
                        ins=[cc2_in[0:QLR // 2, :]], outs=[cc2a_out[:]],
                        replica_groups=[list(range(NC_N))],
                    )
            nc.gpsimd.collective_compute(
                "AllGather", mybir.AluOpType.bypass,
                ins=[cc2_in[QLR // 2:QLR, :]], outs=[cc2b_out[:]],
                replica_groups=[list(range(NC_N))],
            )

            # K^T per head: [DN, S] — overlaps the q AllGathers
            K_t = []
            for i in range(HPC):
                kt = p_K.tile([128, S], bf, tag="K", name="K")
                K_t.append(kt)
                for sc in range(SC):
                    ps = pp_mm.tile([128, 512], f32, tag="mm", name="mm")
                    for c in range(KC):
                        nc.tensor.matmul(ps[:],
                                         kc4[i][:, c * DN:(c + 1) * DN],
                                         ckvg[(c, sc)][:],
                                         start=(c == 0), stop=(c == KC - 1))
                    nc.scalar.activation(kt[:, sc * 512:(sc + 1) * 512],
                                         ps[:], AF.Copy)

            # V natural: per seq-block [128, 4*DV]
            V_t = []
            for sb in range(SB):
                ps = pp_mm.tile([128, 512], f32, tag="mm", name="mm")
                for c in range(KC):
                    nc.tensor.matmul(
                        ps[:],
                        ckvg[(c, sb // 4)][:, (sb % 4) * 128:(sb % 4 + 1) * 128],
                        vc4[:, c * 512:(c + 1) * 512],
                        start=(c == 0), stop=(c == KC - 1))
                vt = p_V.tile([128, HPC * DV], bf, tag="V", name="V")
                nc.scalar.activation(vt[:], ps[:], AF.Copy)
                V_t.append(vt)
        # ---- front pools released here ----

        # ---------------- BACK: head-sharded attention ---------------------
        for sc in range(SC):
            # gathered q_lora^T tiles for this seq chunk (a-half first)
            qlg = []
            for k in range(KQ):
                t = p_qlg.tile([128, 512], bf, tag="qlg", name="qlg")
                buf = cc2a_out if k < KQ // 2 else cc2b_out
                kk = k if k < KQ // 2 else k - KQ // 2
                for half in range(RPC):
                    r = RPC * sc + half
                    nc.sync.dma_start(
                        t[:, half * SL:(half + 1) * SL],
                        buf[r * (QLR // 2) + kk * 128:
                            r * (QLR // 2) + (kk + 1) * 128, :])
                qlg.append(t)
            cosf_t = p_cs.tile([128, 512], bf, tag="cs", name="cs")
            sinf_t = p_cs.tile([128, 512], bf, tag="cs", name="cs")
            nc.sync.dma_start(cosf_t[:], cosf_d[:, sc * 512:(sc + 1) * 512])
            nc.sync.dma_start(sinf_t[:], sinf_d[:, sc * 512:(sc + 1) * 512])
            # Q^T nope per head (transient)
            qn_t = []
            for i in range(HPC):
                ps = pp_mm.tile([128, 512], f32, tag="mm", name="mm")
                for k in range(KQ):
                    nc.tensor.matmul(ps[:], wqb_s(k, i * 128, 128),
                                     qlg[k][:], start=(k == 0),
                                     stop=(k == KQ - 1))
                qt = p_Qn.tile([128, 512], bf, tag="Qn", name="Qn")
                nc.scalar.activation(qt[:], ps[:], AF.Copy)
                qn_t.append(qt)
            # Q^T pe packs + rope (rot = partition swap, sign in sinf)
            roped = []
            for pkt in range(2):
                ps_pe = pp_mm.tile([128, 512], f32, tag="mm", name="mm")
                for k in range(KQ):
                    nc.tensor.matmul(
                        ps_pe[:], wqb_s(k, 512 + pkt * 128, 128),
                        qlg[k][:], start=(k == 0), stop=(k == KQ - 1))
                qpe_sb = p_pe.tile([128, 512], bf, tag="pe", name="pe")
                nc.scalar.activation(qpe_sb[:], ps_pe[:], AF.Copy)
                qrot_sb = p_pe.tile([128, 512], bf, tag="pe", name="pe")
                for h in range(4):
                    src = (h ^ 1) * 32
                    nc.gpsimd.dma_start(qrot_sb[h * 32:(h + 1) * 32, :],
                                        qpe_sb[src:src + 32, :])
                u1 = p_f32.tile([128, 512], f32, tag="rope32", name="rope32")
                u2 = p_f32.tile([128, 512], f32, tag="rope32", name="rope32")
                nc.vector.tensor_mul(u1[:], ps_pe[:], cosf_t[:])
                nc.vector.tensor_mul(u2[:], qrot_sb[:], sinf_t[:])
                rp = p_rope.tile([128, 512], bf, tag="rope", name="rope")
                nc.vector.tensor_add(rp[:], u1[:], u2[:])
                roped.append(rp)

            # attention for each head on this seq chunk
            oT = {}
            for i in range(HPC):
                pkt, hp = i // 2, i % 2
                o_ps = pp_mm.tile([128, 512], f32, tag="mm", name="mm")
                acc = p_acc.tile([128, 512], f32, tag="acc", name="acc")
                nj = 4 * sc + 4
                for j in range(nj):
                    lo = (j - 4 * sc) * 128 if j >= 4 * sc else 0
                    s_ps = pp_mm.tile([128, 512], f32, tag="mm", name="mm")
                    nc.tensor.matmul(s_ps[:, lo:512],
                                     K_t[i][:, j * 128:(j + 1) * 128],
                                     qn_t[i][:, lo:512],
                                     start=True, stop=False)
                    nc.tensor.matmul(
                        s_ps[:, lo:512],
                        kpeg[j // 4][hp * 64:(hp + 1) * 64,
                                     (j % 4) * 128:(j % 4 + 1) * 128],
                        roped[pkt][hp * 64:(hp + 1) * 64, lo:512],
                        start=False, stop=True)
                    if j >= 4 * sc:
                        nc.vector.tensor_add(s_ps[:, lo:lo + 128],
                                             s_ps[:, lo:lo + 128], tri_t[:])
                    pt = p_P.tile([128, 512], bf, tag="P", name="P")
                    nc.scalar.activation(pt[:, lo:512], s_ps[:, lo:512],
                                         AF.Exp)
                    if j == 0:
                        nc.vector.tensor_copy(acc[:], pt[:])
                    else:
                        nc.vector.tensor_add(acc[:, lo:512], acc[:, lo:512],
                                             pt[:, lo:512])
                    nc.tensor.matmul(o_ps[:, lo:512],
                                     V_t[j][:, i * DV:(i + 1) * DV],
                                     pt[:, lo:512],
                                     start=(j == 0), stop=(j == nj - 1),
                                     skip_group_check=True)
                d_ps = pp_sm.tile([1, 512], f32, tag="sm", name="sm")
                nc.tensor.matmul(d_ps[:], ones_f32[:], acc[:],
                                 start=True, stop=True)
                d_sb = p_sml.tile([1, 512], f32, tag="sml", name="sml")
                nc.scalar.activation(d_sb[:], d_ps[:], AF.Copy)
                dbc = p_bc.tile([128, 512], f32, tag="bc", name="bc")
                nc.gpsimd.partition_broadcast(dbc[:], d_sb[:])
                dinv = p_bc.tile([128, 512], f32, tag="bc", name="bc")
                nc.vector.reciprocal(dinv[:], dbc[:])
                ot = p_oT.tile([128, 512], bf, tag="oT", name="oT")
                nc.vector.tensor_mul(ot[:], o_ps[:], dinv[:])
                oT[i] = ot

            # w_o partial for this seq chunk (overlaps later chunks)
            for sbl in range(4):
                sb = sc * 4 + sbl
                stage = p_out.tile([128, HID // 2], bf, tag="outst",
                                   name="outst")
                for n in range(HID // 512):
                    ps = pp_mm.tile([128, 512], f32, tag="mm", name="mm")
                    for i in range(HPC):
                        nc.tensor.matmul(
                            ps[:], oT[i][:, sbl * 128:(sbl + 1) * 128],
                            wo_t[i][:, n * 512:(n + 1) * 512],
                            start=(i == 0), stop=(i == HPC - 1))
                    half, off = n // 5, (n % 5) * 512
                    if n % 2 == 0:
                        nc.vector.tensor_copy(stage[:, off:off + 512], ps[:])
                    else:
                        nc.scalar.activation(stage[:, off:off + 512], ps[:],
                                             AF.Copy)
                    if n % 5 == 4:
                        nc.gpsimd.dma_start(
                            out_d[sb * 128:(sb + 1) * 128,
                                  half * 2560:(half + 1) * 2560],
                            stage[:, :])
                        if n == 4:
                            stage = p_out.tile([128, HID // 2], bf,
                                               tag="outst", name="outst")

    nc.compile()
    return nc


def _prep_inputs(inputs):
    """Host-side sharding + weight folding. Returns in_maps (list of 8 dicts)."""
    BF = _bf16()

    hs = np.asarray(inputs['hidden_states'], np.float32)
    pos = np.asarray(inputs['positions'])
    w_qa = np.asarray(inputs['w_qa'], np.float32)
    q_a_ln_w = np.asarray(inputs['q_a_ln_w'], np.float32)
    w_qb = np.asarray(inputs['w_qb'], np.float32)
    w_kva = np.asarray(inputs['w_kva'], np.float32)
    kv_a_ln_w = np.asarray(inputs['kv_a_ln_w'], np.float32)
    kc = np.asarray(inputs['kc'], np.float32)
    vc = np.asarray(inputs['vc'], np.float32)
    w_o = np.asarray(inputs['w_o'], np.float32)

    perm = np.concatenate([np.arange(0, DR, 2), np.arange(1, DR, 2)])
    inv_freq = 1.0 / (ROPE_BASE ** (np.arange(0, DR, 2, dtype=np.float64) / DR))
    freqs = pos.astype(np.float64)[None, :] * inv_freq[:, None]     # [32, S]
    cosT = np.cos(freqs).astype(np.float32)
    sinT = np.sin(freqs).astype(np.float32)
    cos128 = np.tile(cosT, (4, 1)).astype(BF)                        # [128, S]
    # rot rows carry the swapped halves; the rotation sign lives here:
    # rows 0-31 multiply -sin, rows 32-63 multiply +sin (per 64-block).
    sin128s = np.tile(np.concatenate([-sinT, sinT], 0), (2, 1)).astype(BF)

    scale = DQ ** -0.5
    w_qb_eff = ((w_qb * q_a_ln_w[:, None]) * scale).reshape(QLR, H, DQ)

    w_pe = w_kva[:, KVLR:][:, perm]
    wkvk = np.concatenate([w_kva[:, :KVLR], w_pe, w_pe], 1).astype(BF)

    kc_f = kc * kv_a_ln_w[None, None, :]
    vc_f = vc * kv_a_ln_w[None, :, None]

    kr = np.arange(128)[:, None]
    qc = np.arange(128)[None, :]
    tri = np.where(kr > qc, -1e30, 0.0).astype(np.float32).astype(BF)

    wqa_b = w_qa.astype(BF)

    in_maps = []
    for core in range(NC_N):
        rows = slice(core * SL, (core + 1) * SL)
        h0 = core * HPC

        wqb_all = np.empty((QLR, 768), np.float32)
        for i in range(HPC):
            wqb_all[:, i * 128:(i + 1) * 128] = w_qb_eff[:, h0 + i, :DN]
        for pkt in range(2):
            a, b = h0 + 2 * pkt, h0 + 2 * pkt + 1
            pe_a = w_qb_eff[:, a, DN:][:, perm]
            pe_b = w_qb_eff[:, b, DN:][:, perm]
            wqb_all[:, 512 + pkt * 128:512 + pkt * 128 + 64] = pe_a
            wqb_all[:, 512 + pkt * 128 + 64:512 + (pkt + 1) * 128] = pe_b

        kct = np.concatenate([kc_f[h0 + i].T for i in range(HPC)], 0)
        vcp = np.concatenate([vc_f[h0 + i] for i in range(HPC)], 1)
        wo_sh = w_o[h0 * DV:(h0 + HPC) * DV, :]

        in_maps.append({
            "hsT": np.ascontiguousarray(hs[rows].T).astype(BF),
            "wqa": wqa_b,
            "wkvk": wkvk,
            "cosl": np.ascontiguousarray(cos128[:, rows]),
            "sinl": np.ascontiguousarray(sin128s[:, rows]),
            "cosf": cos128,
            "sinf": sin128s,
            "wqb": wqb_all.astype(BF),
            "kct": kct.astype(BF),
            "vcp": vcp.astype(BF),
            "wo": wo_sh.astype(BF),
            "tri": tri,
        })
    return in_maps


def _get_program():
    if "nc" not in _CACHE:
        _CACHE["nc"] = _build_program()
    return _CACHE["nc"]


def run(inputs, trace=False, trace_kwargs=None):
    """Build (cached), run on 8 cores, return (output, BassKernelResults)."""
    from concourse.bass_utils import run_bass_kernel_spmd

    nc = _get_program()
    in_maps = _prep_inputs(inputs)
    res = run_bass_kernel_spmd(nc, in_maps, list(range(NC_N)),
                               trace=trace, **(trace_kwargs or {}))
    out = np.zeros((S, HID), np.float32)
    for r in res.results:
        out += r["out_partial"].astype(np.float32)
    return out, res


def kernel(**inputs) -> np.ndarray:
    out, _ = run(inputs, trace=False)
    return out
